# revision 8
# baseline (speedup 1.0000x reference)
"""PredRNN (ConvLSTM enc -> BN -> ConvLSTM dec -> BN -> Conv3D -> sigmoid) on 8 trn2 cores.

Sharding: data-parallel over batch (B=8), one sample per core. Per core:
channel-partition layout, 3x3 convs as shift-window matmuls from padded
bf16 image buffers. Encoder h is stored twice per parity tile ([h; h up1]
in partition halves) so ky-taps pair into K=128 matmuls. Enc t>0 runs at
the 5-pass floor per PSUM half: 3 direct pairs (ky0+ky1 taps), one staged
pair (ky2 kx0+kx1 via column-shifted Pool copies), and one staged K=91
matmul merging the ky2 kx2 tap with the 27-row x-conv (x DMA'd straight
into the staging tile). Staging copies are emitted 2 chunks ahead (and
pre-staged across timestep boundaries) so the Pool queue stays in front
of the PE. Decoder uses K=128 fused [e|h_dec] taps (9/chunk, optimal).
PE pass count 9888 = the bf16 structural floor for this layout.
enc(b) and dec(b-2) are interleaved at every timestep: the enc phase
alone is Act-bound (2448 vs 2133 ns/chunk) while dec has PE slack, so
pairing keeps the whole timestep PE-bound; at ts9 the last chunks are
de-interleaved so the dec-only tail drains the eviction backlog before
conv3d WARs on all 8 PSUM banks. ts0 skips all zero-state h convolutions
(enc: x-conv only; dec: 5-pass e-only conv via decw0: 3 pairs + staged
pair + direct ky2 single) and uses a ts0-only gate packing [i|o],[cc|-]
(f unused) so both relu gates evict in one act op. Gates evict from PSUM
with fused hard-sigmoid, all temps bf16, cell state bf16 in SBUF.
DMA-issue placement matters (Pool SWDGE issues cost ~1us of engine time):
dstage writes + border zeroing + D01 prefetch ride SP/HWDGE, weight loads
avoid the Act queue, dstage border zeroing is spread over t=1..8, and the
D01 prefetch is split into row-bands so no 6us transfer monopolizes the
DMA engines. State-image border memsets are deferred into the ts0 loop
except A0 row 0 / B0 (read early or data-hazardous). Conv3D reuses the
dead state buffers via tag-reuse (A1/B0 prefetched during ts9, A0/B1/ct
after), runs q-outer over 8-block PSUM groups on the pA/pB bank
rotations; sigmoid eviction. Final-timestep state writes are skipped.
t_if (the sole pA reader, heading every eviction chain) rotates 3 deep
so PSUM bank recycling decouples from the DVE chain at phase drains.
ts0 is elementwise-bound (no h-convs), so it runs PAIRED: the cc gates of
chunks (2p, 2p+1) share one PSUM bank at opposite partition halves (odd
chunks use swapped [o|i] weight/bias layouts: encxw0o/decw0o/aps cols
12-13) so a single tanh act evicts both, and the ts0-only e-up image copy
is a Pool tensor_copy of the just-written e rows instead of a second DVE
affine - balancing Act/DVE/Pool just above the PE pace.
"""
import sys

sys.path.insert(0, "/opt/trn_rl_repo")
import numpy as np
import ml_dtypes

import concourse.bass as bass
import concourse.tile as tile
from concourse import mybir
from concourse.vector_clock import ScopedClock

BF16 = mybir.dt.bfloat16
F32 = mybir.dt.float32
AF = mybir.ActivationFunctionType
ALU = mybir.AluOpType

T, H, W, F, C = 10, 128, 128, 64, 3
PW = H + 2
NBLK = H // 4
NPIX = H * W
BN_EPS = 1e-3
BFP = ml_dtypes.bfloat16


def _patched_drain_and_barrier(self, tick_clock, wait_clock):
    nc = self.nc
    carrier = nc.sync.nop(nofuse=True, hint="drain_waits")
    wait_clock.add_sem_waits(carrier.ins, ScopedClock({None: tick_clock.global_clock}))
    si = carrier.ins.sync_info
    waits = list(si.on_wait) if si is not None else []
    if len(waits) > 1:
        si.on_wait = waits[:1]
        for w in waits[1:]:
            n = nc.sync.nop(nofuse=True, hint="drain_waits")
            n.ins.sync_info = mybir.SyncInfo(on_wait=[w], on_update=[])
    nc.sync.drain()
    nc.all_engine_barrier()
    popped = nc._tile_sem_poison_stack.pop()
    assert popped is self._sem_poison
    nc.clear_and_free_semaphores(list(self.sems.allocated().values()))
    nc.all_engine_barrier()


tile.TileContext._drain_and_barrier = _patched_drain_and_barrier


def split_multi_waits(nc, max_keep=1):
    """Walrus codegen rejects >1 sem wait on compute instructions; hoist
    extras onto same-engine single-wait NOPs inserted just before."""
    n_split = 0
    for fn in nc.m.functions:
        for blk in fn.blocks:
            insts = blk.instructions
            i = 0
            while i < len(insts):
                inst = insts[i]
                si = inst.sync_info
                waits = list(si.on_wait) if si is not None and si.on_wait else []
                if len(waits) > max_keep:
                    for j, w in enumerate(waits[:-max_keep]):
                        nop = mybir.InstNoOp(
                            name=f"{inst.name}_w{j}",
                            engine=inst.engine,
                            sync_info=mybir.SyncInfo(on_wait=[w], on_update=[]),
                            bass_nofuse=True,
                            ins=[],
                            outs=[],
                        )
                        insts.insert(i, nop)
                        i += 1
                    si.on_wait = waits[-max_keep:]
                    n_split += 1
                i += 1
    return n_split


def _build(do_split=True):
    nc = bass.Bass()
    xim_d = nc.dram_tensor("xim", [T, 27, NPIX], BF16, kind="ExternalInput")
    encw_d = nc.dram_tensor("encw", [128, 5 * 256], BF16, kind="ExternalInput")
    decw0_d = nc.dram_tensor("decw0", [128, 5 * 256], BF16, kind="ExternalInput")
    encxw0_d = nc.dram_tensor("encxw0", [27, 256], BF16, kind="ExternalInput")
    encxw0o_d = nc.dram_tensor("encxw0o", [27, 128], BF16, kind="ExternalInput")
    decw0o_d = nc.dram_tensor("decw0o", [128, 5 * 128], BF16, kind="ExternalInput")
    decw_d = nc.dram_tensor("decw", [128, 9 * 256], BF16, kind="ExternalInput")
    w3_d = nc.dram_tensor("w3", [128, 45 * 30], BF16, kind="ExternalInput")
    aps_d = nc.dram_tensor("aps", [128, 14], F32, kind="ExternalInput")
    bout_d = nc.dram_tensor("bout", [30, 1], F32, kind="ExternalInput")
    dstage = nc.dram_tensor("dstage", [T, F, PW, PW], BF16, kind="Internal")
    y_d = nc.dram_tensor("y", [30, H, W], F32, kind="ExternalOutput")

    with tile.TileContext(nc) as tc:
        with tc.tile_pool(name="wp", bufs=1) as wp:
            encw = wp.tile([128, 5, 256], BF16)
            encxw0 = wp.tile([27, 256], BF16)
            encxw0o = wp.tile([27, 128], BF16)
            decw0o = wp.tile([128, 5, 128], BF16)
            decw = wp.tile([128, 9, 256], BF16)
            decw0 = wp.tile([128, 5, 256], BF16)
            w3t = wp.tile([128, 45, 30], BF16)
            aps = wp.tile([128, 14], F32)
            bout = wp.tile([30, 1], F32)
            zb = wp.tile([64, PW], BF16)
            # ordered by first use: ts0 needs encxw0+aps immediately, decw0 at
            # dec-ts0, encw/decw only from ts1, w3t/bout at conv3d
            nc.sync.dma_start(encxw0[:], encxw0_d[:])
            nc.scalar.dma_start(encxw0o[:], encxw0o_d[:])
            # aps rides the Act queue (it is Act's own first need, ~3us in);
            # decw0 is issued after the first xs loads inside the ts0 loop
            # path (SP queue order: encxw0, xs(0), decw0 via emission below)
            nc.scalar.dma_start(aps[:], aps_d[:])
            nc.gpsimd.dma_start(encw[:], encw_d[:].rearrange("p (s g) -> p s g", s=5))
            nc.gpsimd.dma_start(decw[:], decw_d[:].rearrange("p (s g) -> p s g", s=9))
            nc.gpsimd.dma_start(w3t[:], w3_d[:].rearrange("p (s g) -> p s g", s=45))
            nc.gpsimd.dma_start(bout[:], bout_d[:])
            nc.vector.memset(zb[:], 0.0)

            with tc.tile_pool(name="sp", bufs=1) as sp, \
                 tc.tile_pool(name="tp", bufs=1) as tp, \
                 tc.tile_pool(name="pp", bufs=1, space=bass.MemorySpace.PSUM) as pp:
                # state tiles carry tags so conv3d can reuse their buffers
                # (tag-reuse = same space, deps tracked) once they go dead
                A0 = sp.tile([128, PW, PW], BF16, tag="A0", name="A0")
                A1 = sp.tile([128, PW, PW], BF16, tag="A1", name="A1")
                B0 = sp.tile([128, PW, PW], BF16, tag="B0", name="B0")
                B1 = sp.tile([128, PW, PW], BF16, tag="B1", name="B1")
                # cell state allocated at PW*PW (not NPIX) so its buffer can
                # host a conv3d frame pair later; only [:, :NPIX] is used
                ct = sp.tile([128, PW * PW], BF16, tag="ct", name="ct")
                # ts0 skips all h-reads, so only the never-written borders of
                # the state images need zeroing (pad rows/cols + bottom row
                # 128). B0 first: dec-ts0 reads it within ~10us; the other
                # tiles aren't read before t=1, so their memsets are emitted
                # lazily inside the ts0 loop (see below) to keep the DVE
                # queue clear for the first evictions.
                # on Pool (idle at startup; DVE would delay the first
                # eviction chains). Pool memset runs at full efficiency.
                for im in (B0,):
                    nc.gpsimd.memset(im[:, 0, :], 0.0)
                    nc.gpsimd.memset(im[:, 128:130, :], 0.0)
                    nc.gpsimd.memset(im[:, :, 0], 0.0)
                    nc.gpsimd.memset(im[:, :, PW - 1], 0.0)
                # A0's bottom-half row 0 is DATA (enc(0,b=0)'s rowup copy
                # writes rows 0:4): this memset must precede that write, so
                # it cannot be deferred with the other lazy strips
                nc.gpsimd.memset(A0[:, 0, :], 0.0)
                # ct needs no memset: every element is written at ts0 (c0 = i*cc)
                # before any read. dstage border zeroing is spread over
                # t=1..8 on SP (only needs to precede the conv3d reads).
                Bs = [B0, B1]
                As = [A0, A1]

                def tmp(tag):
                    # t_if heads every eviction chain (sole pA reader): give
                    # it a deeper rotation so its WAR trails 3 chunk-pairs
                    # instead of 2, decoupling PSUM recycling at drains
                    return tp.tile([128, 512], BF16, tag=tag,
                                   bufs=(3 if tag == "t_if" else 2), name=tag)

                def enc_stage(b, t, Aprev):
                    # ky2 taps relocated/column-shifted into K=128-packable
                    # tiles; x rides partitions 64:91 of s2 (K=91 pass).
                    # Aprev bottom half is h stored rowup: window rows
                    # r0+1:r0+5 = tap ky2, col offset = kx.
                    r0, c0 = 4 * b, 512 * b
                    s1 = tp.tile([128, 4, 128], BF16, tag="s1", bufs=4)
                    s2 = tp.tile([128, 4, 128], BF16, tag="s2", bufs=4)
                    nc.gpsimd.tensor_copy(s1[0:64, :, :], Aprev[64:128, r0 + 1:r0 + 5, 0:128])
                    nc.gpsimd.tensor_copy(s1[64:128, :, :], Aprev[64:128, r0 + 1:r0 + 5, 1:129])
                    nc.gpsimd.tensor_copy(s2[0:64, :, :], Aprev[64:128, r0 + 1:r0 + 5, 2:130])
                    nc.sync.dma_start(
                        s2[64:91, :, :],
                        xim_d[t, :, 512 * b:512 * b + 512].rearrange("p (a b) -> p a b", a=4))
                    return s1, s2

                D01 = []
                for t in range(T):
                    Acur, Aprev = As[t % 2], As[(t - 1) % 2]
                    Bcur, Bnext = Bs[t % 2], Bs[(t + 1) % 2]

                    # ---------------- encoder ----------------
                    def enc_block(b, stage=None, t=t, Acur=Acur, Aprev=Aprev, Bcur=Bcur):
                        r0, c0 = 4 * b, 512 * b
                        if t == 0:
                            xs = tp.tile([27, 512], BF16, tag="s1", bufs=4)
                            nc.sync.dma_start(xs[:], xim_d[t, :, c0:c0 + 512])
                        else:
                            s1, s2 = stage
                        pA = pp.tile([128, 512], F32, tag="pA", bufs=4)
                        pB = pp.tile([128, 512], F32, tag="pB", bufs=4)
                        for ch, ps in ((0, pA), (1, pB)):
                            if t > 0:  # h_{-1}=0: ts0 needs only the x conv
                                for kx in range(3):
                                    # K=128 pair: ky=0 (top) + ky=1 (bottom)
                                    nc.tensor.matmul(
                                        ps[:],
                                        encw[:, kx, 128 * ch:128 * ch + 128],
                                        Aprev[:, r0:r0 + 4, kx:kx + 128],
                                        start=(kx == 0), stop=False)
                                # K=128 staged pair: taps (2,0)+(2,1)
                                nc.tensor.matmul(
                                    ps[:], encw[:, 3, 128 * ch:128 * ch + 128],
                                    s1[:, :, :], start=False, stop=False)
                                # K=91: tap (2,2) + 27-row x-conv
                                nc.tensor.matmul(
                                    ps[:], encw[0:91, 4, 128 * ch:128 * ch + 128],
                                    s2[0:91, :, :], start=False, stop=True)
                            else:
                                nc.tensor.matmul(
                                    ps[:], encxw0[:, 128 * ch:128 * ch + 128], xs[:],
                                    start=True, stop=True)
                        t_if, t_tc, t_o = tmp("t_if"), tmp("t_tc"), tmp("t_o")
                        t_s, t_s2, t_th, t_h = tmp("t_s"), tmp("t_s2"), tmp("t_th"), tmp("t_h")
                        cblk = ct[64:128, c0:c0 + 512]
                        if t == 0:
                            # ts0: f unused (c_{-1}=0); gates packed [i|o],[cc|-]
                            # so both relu gates evict in ONE act op
                            nc.scalar.activation(t_if[:], pA[:], AF.Relu, bias=aps[:, 8:9], scale=0.2)
                            nc.scalar.activation(t_tc[0:64, :], pB[0:64, :], AF.Tanh, bias=aps[0:64, 9:10], scale=1.0)
                            nc.vector.scalar_tensor_tensor(cblk, t_if[0:64, :], 1.0, t_tc[0:64, :], ALU.min, ALU.mult)
                            # t_th on partitions 64:128 so the STT inputs
                            # (o-gate at 64:128) share a base partition
                            nc.scalar.activation(t_th[64:128, :], cblk, AF.Tanh)
                            nc.vector.scalar_tensor_tensor(t_h[0:64, :], t_if[64:128, :], 1.0, t_th[64:128, :], ALU.min, ALU.mult)
                        else:
                            nc.scalar.activation(t_if[:], pA[:], AF.Relu, bias=aps[:, 0:1], scale=0.2)
                            nc.scalar.activation(t_tc[0:64, :], pB[0:64, :], AF.Tanh, bias=aps[0:64, 2:3], scale=1.0)
                            nc.scalar.activation(t_o[0:64, :], pB[64:128, :], AF.Relu, bias=aps[64:128, 2:3], scale=0.2)
                            nc.vector.scalar_tensor_tensor(t_s[64:128, :], t_if[0:64, :], 1.0, t_tc[0:64, :], ALU.min, ALU.mult)
                            nc.vector.scalar_tensor_tensor(t_s2[64:128, :], t_if[64:128, :], 1.0, cblk, ALU.min, ALU.mult)
                            nc.vector.tensor_tensor(cblk, t_s[64:128, :], t_s2[64:128, :], ALU.add)
                            nc.scalar.activation(t_th[0:64, :], cblk, AF.Tanh)
                            nc.vector.scalar_tensor_tensor(t_h[0:64, :], t_o[0:64, :], 1.0, t_th[0:64, :], ALU.min, ALU.mult)
                        hr = t_h[0:64, :].rearrange("p (a b) -> p a b", a=4)
                        if t < T - 1:  # ts9's h_enc is never convolved again
                            nc.gpsimd.tensor_copy(Acur[0:64, r0 + 1:r0 + 5, 1:1 + W], hr)
                            nc.gpsimd.tensor_copy(Acur[64:128, r0:r0 + 4, 1:1 + W], hr)
                        nc.vector.tensor_scalar(
                            Bcur[0:64, r0 + 1:r0 + 5, 1:1 + W], hr,
                            aps[0:64, 4:5], aps[0:64, 5:6], ALU.mult, ALU.add)
                        if t == 0:  # also e down1 into h-half for paired dec ts0
                            nc.vector.tensor_scalar(
                                Bcur[64:128, r0:r0 + 4, 1:1 + W], hr,
                                aps[0:64, 4:5], aps[0:64, 5:6], ALU.mult, ALU.add)

                    def dec_stage0(b, Bcur=Bcur):
                        # ts0 e-only conv: stage the (2,0)+(2,1) pair from the
                        # e-up half of B (column shifts baked into the copies)
                        r0 = 4 * b
                        s1 = tp.tile([128, 4, 128], BF16, tag="s2", bufs=4)
                        nc.gpsimd.tensor_copy(s1[0:64, :, :], Bcur[64:128, r0 + 1:r0 + 5, 0:128])
                        nc.gpsimd.tensor_copy(s1[64:128, :, :], Bcur[64:128, r0 + 1:r0 + 5, 1:129])
                        return s1

                    # ---------------- decoder ----------------
                    def dec_block(b, stage=None, t=t, Bcur=Bcur, Bnext=Bnext):
                        r0, c0 = 4 * b, 512 * b
                        pA = pp.tile([128, 512], F32, tag="pA", bufs=4)
                        pB = pp.tile([128, 512], F32, tag="pB", bufs=4)
                        for ch, ps in ((0, pA), (1, pB)):
                            if t == 0:
                                # h_dec_{-1}=0; Bcur holds [e; e up1] -> pair kys
                                for kx in range(3):
                                    nc.tensor.matmul(
                                        ps[:],
                                        decw0[:, kx, 128 * ch:128 * ch + 128],
                                        Bcur[:, r0:r0 + 4, kx:kx + 128],
                                        start=(kx == 0), stop=False)
                                # staged pair (2,0)+(2,1)
                                nc.tensor.matmul(
                                    ps[:], decw0[:, 3, 128 * ch:128 * ch + 128],
                                    stage[:, :, :], start=False, stop=False)
                                # direct K=64 single (2,2)
                                nc.tensor.matmul(
                                    ps[:], decw0[64:128, 4, 128 * ch:128 * ch + 128],
                                    Bcur[64:128, r0 + 1:r0 + 5, 2:130],
                                    start=False, stop=True)
                            else:
                                for s in range(9):
                                    ky, kx = s // 3, s % 3
                                    nc.tensor.matmul(
                                        ps[:],
                                        decw[:, s, 128 * ch:128 * ch + 128],
                                        Bcur[:, r0 + ky:r0 + ky + 4, kx:kx + 128],
                                        start=(s == 0), stop=(s == 8))
                        t_if, t_tc, t_o = tmp("t_if"), tmp("t_tc"), tmp("t_o")
                        t_s, t_s2, t_th, t_h = tmp("t_s"), tmp("t_s2"), tmp("t_th"), tmp("t_h")
                        cblk = ct[0:64, c0:c0 + 512]
                        if t == 0:
                            # ts0: f unused; decw0 gates packed [i|o],[cc|-]
                            # so both relu gates evict in ONE act op
                            nc.scalar.activation(t_if[:], pA[:], AF.Relu, bias=aps[:, 10:11], scale=0.2)
                            nc.scalar.activation(t_tc[0:64, :], pB[0:64, :], AF.Tanh, bias=aps[0:64, 11:12], scale=1.0)
                            nc.vector.scalar_tensor_tensor(cblk, t_if[0:64, :], 1.0, t_tc[0:64, :], ALU.min, ALU.mult)
                            nc.scalar.activation(t_th[64:128, :], cblk, AF.Tanh)
                            nc.vector.scalar_tensor_tensor(t_h[0:64, :], t_if[64:128, :], 1.0, t_th[64:128, :], ALU.min, ALU.mult)
                        else:
                            # chunk A is [f|i] (host-permuted columns)
                            nc.scalar.activation(t_if[:], pA[:], AF.Relu, bias=aps[:, 1:2], scale=0.2)
                            nc.scalar.activation(t_tc[64:128, :], pB[0:64, :], AF.Tanh, bias=aps[0:64, 3:4], scale=1.0)
                            nc.scalar.activation(t_o[0:64, :], pB[64:128, :], AF.Relu, bias=aps[64:128, 3:4], scale=0.2)
                            nc.vector.scalar_tensor_tensor(t_s2[0:64, :], t_if[0:64, :], 1.0, cblk, ALU.min, ALU.mult)
                            nc.vector.scalar_tensor_tensor(t_s[0:64, :], t_if[64:128, :], 1.0, t_tc[64:128, :], ALU.min, ALU.mult)
                            nc.vector.tensor_tensor(cblk, t_s[0:64, :], t_s2[0:64, :], ALU.add)
                            nc.scalar.activation(t_th[0:64, :], cblk, AF.Tanh)
                            nc.vector.scalar_tensor_tensor(t_h[0:64, :], t_o[0:64, :], 1.0, t_th[0:64, :], ALU.min, ALU.mult)
                        hr = t_h[0:64, :].rearrange("p (a b) -> p a b", a=4)
                        if t < T - 1:  # ts9's h_dec feeds no further timestep
                            nc.gpsimd.tensor_copy(Bnext[64:128, r0 + 1:r0 + 5, 1:1 + W], hr)
                        dtmp = tp.tile([64, 512], BF16, tag="dtmp", bufs=2)
                        nc.vector.tensor_scalar(
                            dtmp[:], t_h[0:64, :],
                            aps[0:64, 6:7], aps[0:64, 7:8], ALU.mult, ALU.add)
                        # SP-issued (HWDGE ~650ns) instead of Pool (SWDGE ~1us):
                        # keeps the Pool queue free for staging + h-writes
                        nc.sync.dma_start(
                            dstage[t, :, r0 + 1:r0 + 5, 1:1 + W],
                            dtmp[:].rearrange("p (a b) -> p a b", a=4))

                    # ---------------- ts0 pair blocks ----------------
                    # cc gates of a chunk pair share ONE PSUM bank at opposite
                    # partition halves (odd chunks use swapped [o|i] weights +
                    # bias columns), so a single 612ns tanh evicts both ccs.
                    # Even chunks' c-STT runs on Pool to balance DVE.
                    def enc_pair0(p, Acur=Acur, Bcur=Bcur):
                        pB = pp.tile([128, 512], F32, tag="pB", bufs=4)
                        gates = []
                        for j in (0, 1):
                            b = 2 * p + j
                            c0 = 512 * b
                            xs = tp.tile([27, 512], BF16, tag="s1", bufs=4)
                            nc.sync.dma_start(xs[:], xim_d[0, :, c0:c0 + 512])
                            pA = pp.tile([128, 512], F32, tag="pA", bufs=4)
                            nc.tensor.matmul(
                                pA[:],
                                encxw0[:, 0:128] if j == 0 else encxw0o[:],
                                xs[:], start=True, stop=True)
                            nc.tensor.matmul(
                                pB[64 * j:64 * j + 64, :], encxw0[:, 128:192],
                                xs[:], start=True, stop=True)
                            t_if = tmp("t_if")
                            nc.scalar.activation(
                                t_if[:], pA[:], AF.Relu,
                                bias=aps[:, 8 + 4 * j:9 + 4 * j], scale=0.2)
                            gates.append(t_if)
                        t_tc = tmp("t_tc")
                        nc.scalar.activation(t_tc[:], pB[:], AF.Tanh,
                                             bias=aps[:, 9:10], scale=1.0)
                        for j in (0, 1):
                            b = 2 * p + j
                            r0, c0 = 4 * b, 512 * b
                            t_if = gates[j]
                            cblk = ct[64:128, c0:c0 + 512]
                            i_sl = slice(64 * j, 64 * j + 64)
                            o_sl = slice(64 - 64 * j, 128 - 64 * j)
                            nc.vector.scalar_tensor_tensor(
                                cblk, t_if[i_sl, :], 1.0, t_tc[i_sl, :],
                                ALU.min, ALU.mult)
                            t_th, t_h = tmp("t_th"), tmp("t_h")
                            nc.scalar.activation(t_th[o_sl, :], cblk, AF.Tanh)
                            nc.vector.scalar_tensor_tensor(
                                t_h[0:64, :], t_if[o_sl, :], 1.0, t_th[o_sl, :],
                                ALU.min, ALU.mult)
                            hr = t_h[0:64, :].rearrange("p (a b) -> p a b", a=4)
                            nc.gpsimd.tensor_copy(Acur[0:64, r0 + 1:r0 + 5, 1:1 + W], hr)
                            nc.gpsimd.tensor_copy(Acur[64:128, r0:r0 + 4, 1:1 + W], hr)
                            nc.vector.tensor_scalar(
                                Bcur[0:64, r0 + 1:r0 + 5, 1:1 + W], hr,
                                aps[0:64, 4:5], aps[0:64, 5:6], ALU.mult, ALU.add)
                            # e-up = copy of the e rows just written (Pool,
                            # not a second DVE affine: DVE is the ts0 binder)
                            nc.gpsimd.tensor_copy(
                                Bcur[64:128, r0:r0 + 4, 1:1 + W],
                                Bcur[0:64, r0 + 1:r0 + 5, 1:1 + W])

                    def dec_pair0(p, st2, Bcur=Bcur, Bnext=Bnext):
                        pB = pp.tile([128, 512], F32, tag="pB", bufs=4)
                        gates = []
                        for j in (0, 1):
                            b = 2 * p + j
                            r0 = 4 * b
                            st = st2[j]
                            pA = pp.tile([128, 512], F32, tag="pA", bufs=4)
                            wA = decw0 if j == 0 else decw0o
                            for kx in range(3):
                                nc.tensor.matmul(
                                    pA[:], wA[:, kx, 0:128],
                                    Bcur[:, r0:r0 + 4, kx:kx + 128],
                                    start=(kx == 0), stop=False)
                                nc.tensor.matmul(
                                    pB[64 * j:64 * j + 64, :], decw0[:, kx, 128:192],
                                    Bcur[:, r0:r0 + 4, kx:kx + 128],
                                    start=(kx == 0), stop=False)
                            nc.tensor.matmul(
                                pA[:], wA[:, 3, 0:128], st[:, :, :],
                                start=False, stop=False)
                            nc.tensor.matmul(
                                pB[64 * j:64 * j + 64, :], decw0[:, 3, 128:192],
                                st[:, :, :], start=False, stop=False)
                            nc.tensor.matmul(
                                pA[:], wA[64:128, 4, 0:128],
                                Bcur[64:128, r0 + 1:r0 + 5, 2:130],
                                start=False, stop=True)
                            nc.tensor.matmul(
                                pB[64 * j:64 * j + 64, :], decw0[64:128, 4, 128:192],
                                Bcur[64:128, r0 + 1:r0 + 5, 2:130],
                                start=False, stop=True)
                            t_if = tmp("t_if")
                            nc.scalar.activation(
                                t_if[:], pA[:], AF.Relu,
                                bias=aps[:, 10 + 3 * j:11 + 3 * j], scale=0.2)
                            gates.append(t_if)
                        t_tc = tmp("t_tc")
                        nc.scalar.activation(t_tc[:], pB[:], AF.Tanh,
                                             bias=aps[:, 11:12], scale=1.0)
                        for j in (0, 1):
                            b = 2 * p + j
                            r0, c0 = 4 * b, 512 * b
                            t_if = gates[j]
                            cblk = ct[0:64, c0:c0 + 512]
                            i_sl = slice(64 * j, 64 * j + 64)
                            o_sl = slice(64 - 64 * j, 128 - 64 * j)
                            nc.vector.scalar_tensor_tensor(
                                cblk, t_if[i_sl, :], 1.0, t_tc[i_sl, :],
                                ALU.min, ALU.mult)
                            t_th, t_h = tmp("t_th"), tmp("t_h")
                            nc.scalar.activation(t_th[o_sl, :], cblk, AF.Tanh)
                            nc.vector.scalar_tensor_tensor(
                                t_h[0:64, :], t_if[o_sl, :], 1.0, t_th[o_sl, :],
                                ALU.min, ALU.mult)
                            hr = t_h[0:64, :].rearrange("p (a b) -> p a b", a=4)
                            nc.gpsimd.tensor_copy(Bnext[64:128, r0 + 1:r0 + 5, 1:1 + W], hr)
                            dtmp = tp.tile([64, 512], BF16, tag="dtmp", bufs=2)
                            nc.vector.tensor_scalar(
                                dtmp[:], t_h[0:64, :],
                                aps[0:64, 6:7], aps[0:64, 7:8], ALU.mult, ALU.add)
                            nc.sync.dma_start(
                                dstage[0, :, r0 + 1:r0 + 5, 1:1 + W],
                                dtmp[:].rearrange("p (a b) -> p a b", a=4))

                    if t == 0:
                        # coarse interleave: dec blocks (PE-heavy) fill the
                        # PE while enc evictions (act-paced) run; offset 4
                        # keeps dec eviction chains from head-blocking the
                        # act queue (dec(k) needs enc(k+1)'s bottom row)
                        dstages = {}
                        # A0/A1/B1 borders aren't read before t=1: emit their
                        # memsets lazily mid-ts0 to keep the DVE queue clear
                        # for the first eviction chains
                        lazy_ms = []
                        for im in (A0, A1, B1):
                            lazy_ms += [
                                lambda im=im: nc.vector.memset(im[:, 128:130, :], 0.0),
                                lambda im=im: nc.vector.memset(im[:, :, 0], 0.0),
                                lambda im=im: nc.vector.memset(im[:, :, PW - 1], 0.0),
                            ]
                            if im is not A0:  # A0 row 0 is set upfront (data hazard)
                                lazy_ms.append(
                                    lambda im=im: nc.vector.memset(im[:, 0, :], 0.0))
                        NP = NBLK // 2
                        for p in range(NP):
                            enc_pair0(p)
                            if p == 0:
                                # after the first xs loads in the SP queue;
                                # dec pair 0 needs these only at ~10us
                                nc.sync.dma_start(
                                    decw0[:],
                                    decw0_d[:].rearrange("p (s g) -> p s g", s=5))
                                nc.sync.dma_start(
                                    decw0o[:],
                                    decw0o_d[:].rearrange("p (s g) -> p s g", s=5))
                            if 3 <= p < 3 + len(lazy_ms):
                                lazy_ms[p - 3]()
                            if p >= 1:
                                dstages[2 * p - 1] = dec_stage0(2 * p - 1)
                            if p + 1 < NP:
                                # chunk 2p needs only enc chunk 2p+1 (this
                                # pair): emit before next pair's Pool writes
                                dstages[2 * p] = dec_stage0(2 * p)
                            if p >= 2:
                                k = 2 * (p - 2)
                                dec_pair0(p - 2, [dstages.pop(k), dstages.pop(k + 1)])
                        # tail: stage emissions interleaved with consumers so
                        # slot reuse never head-blocks the Pool queue
                        dstages[NBLK - 2] = dec_stage0(NBLK - 2)
                        dec_pair0(NP - 2, [dstages.pop(NBLK - 4), dstages.pop(NBLK - 3)])
                        dstages[NBLK - 1] = dec_stage0(NBLK - 1)
                        dec_pair0(NP - 1, [dstages.pop(NBLK - 2), dstages.pop(NBLK - 1)])
                        # pre-stage t=1 chunks 0/1 so enc(1,0) starts clean
                        pend = {0: enc_stage(0, 1, Acur), 1: enc_stage(1, 1, Acur)}
                        continue
                    # dstage borders for earlier frames (conv3d needs them;
                    # nothing reads them before ts9): ~1 frame per timestep
                    bframes = [t - 1] if t < 8 else ([7, 8, 9] if t == 8 else [])
                    for bf in bframes:
                        nc.sync.dma_start(dstage[bf, :, 0, :], zb[:])
                        nc.sync.dma_start(dstage[bf, :, PW - 1, :], zb[:])
                        nc.sync.dma_start(dstage[bf, :, :, 0], zb[:])
                        nc.sync.dma_start(dstage[bf, :, :, PW - 1], zb[:])
                    stages = pend
                    if t == 1:
                        # reclaim B0 bottom row 0 (junked by ts0's e-down1);
                        # dec(1) writes B0 rows 1..128 only
                        nc.vector.memset(Bnext[64:128, 0, :], 0.0)
                    if t == T - 1:
                        # A1/B0 are dead through ts9 (final-ts writes
                        # skipped, last reads at ts8): prefetch conv3d
                        # frames 0..3 into their buffers during dec ts9.
                        # Split into row-bands so no single 6us transfer
                        # monopolizes the DMA engines against the small
                        # latency-critical stage/dstage transfers.
                        D01 = [
                            sp.tile([128, PW, PW], BF16, tag="A1", name="Dp0"),
                            sp.tile([128, PW, PW], BF16, tag="B0", name="Dp1"),
                        ]
                        d01_parts = []
                        nband = 4
                        rb = [0, 33, 66, 99, PW]
                        for q in range(2):
                            for hh, h0 in ((0, 0), (64, 1)):
                                for k in range(nband):
                                    d01_parts.append((q, hh, rb[k], rb[k + 1], h0))
                    # interleave enc(b) with dec(b-2): the enc phase alone is
                    # act-bound (2448 > 2133 ns/chunk) while dec has PE slack;
                    # pairing keeps every chunk-pair PE-bound. dec(k) needs
                    # e rows through 4k+4, written by enc(k+1).
                    def dec_k(k, t=t):
                        if t == T - 1 and k < len(d01_parts):
                            q, hh, ra, rz, h0 = d01_parts[k]
                            nc.sync.dma_start(
                                D01[q][hh:hh + 64, ra:rz, :],
                                dstage[2 * q + h0, :, ra:rz, :])
                        dec_block(k)
                    # at t==T-1, de-interleave the last chunks: the dec-only
                    # tail has PE slack (3840 vs 2448 ns/chunk), letting the
                    # Act/DVE eviction backlog drain before conv3d WARs on
                    # all 8 PSUM banks (else ~10us PE stall at conv3d start)
                    ilv_last = NBLK - 2 if t < T - 1 else 28
                    for b in range(NBLK):
                        if b + 2 < NBLK:
                            stages[b + 2] = enc_stage(b + 2, t, Aprev)
                        enc_block(b, stages.pop(b))
                        if 0 <= b - 2 < ilv_last:
                            dec_k(b - 2)
                    for k in range(ilv_last, NBLK):
                        dec_k(k)
                        if t < T - 1:
                            # pre-stage the next timestep's first two chunks
                            # during the dec tail so enc(t+1,0) starts clean
                            pend = pend if k > NBLK - 2 else {}
                            pend[k - (NBLK - 2)] = enc_stage(
                                k - (NBLK - 2), t + 1, Acur)

                # ---------------- conv3d + sigmoid ----------------
                # frames 0..3 prefetched during ts9 (D01); frames 4..9 load
                # into the now-dead A0/B1/ct buffers via tag reuse
                D = D01 + [
                    sp.tile([128, PW, PW], BF16, tag="A0", name="D2"),
                    sp.tile([128, PW, PW], BF16, tag="B1", name="D3"),
                    sp.tile([128, PW, PW], BF16, tag="ct", name="D4"),
                ]
                dma_engs = [nc.sync, nc.scalar, nc.gpsimd]
                for q in range(2, 5):
                    dma_engs[(2 * q) % 3].dma_start(
                        D[q][0:64, :, :], dstage[2 * q, :, :, :])
                    dma_engs[(2 * q + 1) % 3].dma_start(
                        D[q][64:128, :, :], dstage[2 * q + 1, :, :, :])
                # q-outer over 8-block groups: early matmuls need only D0
                # while later D tiles are still in flight; PSUM groups reuse
                # the pA/pB bank rotations (rows 0:30 of each bank)
                for g in range(NBLK // 8):
                    pys = [pp.tile([128, 512], F32,
                                   tag=("pA" if bb % 2 == 0 else "pB"), bufs=4,
                                   name=f"py{bb}") for bb in range(8)]
                    for q in range(5):
                        for bb in range(8):
                            r0 = 4 * (8 * g + bb)
                            for s in range(9):
                                ky, kx = s // 3, s % 3
                                nc.tensor.matmul(
                                    pys[bb][0:30, :], w3t[:, q * 9 + s, :],
                                    D[q][:, r0 + ky:r0 + ky + 4, kx:kx + 128],
                                    start=(q == 0 and s == 0),
                                    stop=(q == 4 and s == 8))
                    for bb in range(8):
                        r0 = 4 * (8 * g + bb)
                        ty = tp.tile([30, 512], F32, tag="ty", bufs=1)
                        nc.scalar.activation(ty[:], pys[bb][0:30, :], AF.Sigmoid,
                                             bias=bout[:], scale=1.0)
                        nc.scalar.dma_start(
                            y_d[:, r0:r0 + 4, :],
                            ty[:].rearrange("p (a b) -> p a b", a=4))

    if do_split:
        split_multi_waits(nc)
    nc.finalize()
    return nc


def _prep(inputs):
    x = np.asarray(inputs["x"], np.float32)
    xpad = np.zeros((8, T, PW, PW, C), np.float32)
    xpad[:, :, 1:1 + H, 1:1 + W, :] = x
    xim = np.empty((8, T, 27, NPIX), BFP)
    for ky in range(3):
        for kx in range(3):
            s = ky * 3 + kx
            v = xpad[:, :, ky:ky + H, kx:kx + W, :]
            xim[:, :, s * 3:s * 3 + 3, :] = (
                v.transpose(0, 1, 4, 2, 3).reshape(8, T, 3, NPIX).astype(BFP))

    enc_Wh = np.asarray(inputs["enc_Wh"], np.float32)
    enc_Wx = np.asarray(inputs["enc_Wx"], np.float32)
    dec_Wx = np.asarray(inputs["dec_Wx"], np.float32)
    dec_Wh = np.asarray(inputs["dec_Wh"], np.float32)
    out_W = np.asarray(inputs["out_W"], np.float32)

    encw = np.zeros((128, 5, 256), np.float32)
    decw = np.zeros((128, 9, 256), np.float32)
    perm = np.concatenate([np.arange(64, 128), np.arange(0, 64), np.arange(128, 256)])
    perm0 = np.concatenate([np.arange(0, 64), np.arange(192, 256),
                            np.arange(128, 192), np.arange(64, 128)])
    for s in range(9):
        ky, kx = s // 3, s % 3
        decw[0:64, s, :] = dec_Wx[ky, kx][:, perm]
        decw[64:128, s, :] = dec_Wh[ky, kx][:, perm]
    decw0 = np.zeros((128, 5, 256), np.float32)
    for kx in range(3):
        # paired matmul: top half = tap ky=0, bottom = tap ky=1 (e up1 copy)
        encw[0:64, kx, :] = enc_Wh[0, kx]
        encw[64:128, kx, :] = enc_Wh[1, kx]
        # ts0 decoder: e-only paired conv, gates packed [i|o|c|junk]
        decw0[0:64, kx, :] = dec_Wx[0, kx][:, perm0]
        decw0[64:128, kx, :] = dec_Wx[1, kx][:, perm0]
    # staged pair (2,0)+(2,1) and merged (2,2)+x slots
    encw[0:64, 3, :] = enc_Wh[2, 0]
    encw[64:128, 3, :] = enc_Wh[2, 1]
    encw[0:64, 4, :] = enc_Wh[2, 2]
    encw[64:91, 4, :] = enc_Wx.reshape(27, 256)
    decw0[0:64, 3, :] = dec_Wx[2, 0][:, perm0]
    decw0[64:128, 3, :] = dec_Wx[2, 1][:, perm0]
    decw0[64:128, 4, :] = dec_Wx[2, 2][:, perm0]
    encxw = enc_Wx.reshape(27, 256)

    w3 = np.zeros((45, 128, 30), np.float32)
    for q in range(5):
        for j in range(2):
            f = 2 * q + j
            for t in range(max(0, f - 1), min(T - 1, f + 1) + 1):
                dt = f - t + 1
                for s in range(9):
                    ky, kx = s // 3, s % 3
                    w3[q * 9 + s, 64 * j:64 * j + 64, 3 * t:3 * t + 3] = out_W[dt, ky, kx]
    w3 = w3.transpose(1, 0, 2)  # [128, 45, 30]

    enc_b = np.asarray(inputs["enc_b"], np.float32)
    dec_b = np.asarray(inputs["dec_b"], np.float32)
    s_e = np.asarray(inputs["enc_gamma"], np.float32) / np.sqrt(
        np.asarray(inputs["enc_var"], np.float32) + BN_EPS)
    t_e = np.asarray(inputs["enc_beta"], np.float32) - np.asarray(inputs["enc_mean"], np.float32) * s_e
    s_d = np.asarray(inputs["dec_gamma"], np.float32) / np.sqrt(
        np.asarray(inputs["dec_var"], np.float32) + BN_EPS)
    t_d = np.asarray(inputs["dec_beta"], np.float32) - np.asarray(inputs["dec_mean"], np.float32) * s_d

    # ts0 encoder x-conv with gate columns [i|o|c|0] (f unused at ts0)
    encxw0 = np.zeros((27, 256), np.float32)
    encxw0[:, 0:64] = encxw[:, 0:64]
    encxw0[:, 64:128] = encxw[:, 192:256]
    encxw0[:, 128:192] = encxw[:, 128:192]

    aps = np.zeros((128, 14), np.float32)
    aps[0:64, 0] = 0.2 * enc_b[0:64] + 0.5          # enc i
    aps[64:128, 0] = 0.2 * enc_b[64:128] + 0.5      # enc f
    aps[0:64, 1] = 0.2 * dec_b[64:128] + 0.5        # dec f (chunk A is [f|i])
    aps[64:128, 1] = 0.2 * dec_b[0:64] + 0.5        # dec i
    aps[0:64, 2] = enc_b[128:192]                   # enc c~ (tanh bias)
    aps[64:128, 2] = 0.2 * enc_b[192:256] + 0.5     # enc o
    aps[0:64, 3] = dec_b[128:192]
    aps[64:128, 3] = 0.2 * dec_b[192:256] + 0.5
    aps[0:64, 4] = s_e
    aps[0:64, 5] = t_e
    aps[0:64, 6] = s_d
    aps[0:64, 7] = t_d
    aps[0:64, 8] = 0.2 * enc_b[0:64] + 0.5          # ts0 enc i
    aps[64:128, 8] = 0.2 * enc_b[192:256] + 0.5     # ts0 enc o
    aps[0:64, 9] = enc_b[128:192]                   # ts0 enc c~ (tanh bias)
    aps[0:64, 10] = 0.2 * dec_b[0:64] + 0.5         # ts0 dec i
    aps[64:128, 10] = 0.2 * dec_b[192:256] + 0.5    # ts0 dec o
    aps[0:64, 11] = dec_b[128:192]                  # ts0 dec c~ (tanh bias)
    # ts0 pair-packing: cc tanh acts cover both partition halves (odd chunk's
    # cc lands at 64:128), and odd chunks use swapped [o|i] gate layouts
    aps[64:128, 9] = enc_b[128:192]                 # ts0 enc c~ odd half
    aps[64:128, 11] = dec_b[128:192]                # ts0 dec c~ odd half
    aps[0:64, 12] = 0.2 * enc_b[192:256] + 0.5      # ts0 enc odd o
    aps[64:128, 12] = 0.2 * enc_b[0:64] + 0.5       # ts0 enc odd i
    aps[0:64, 13] = 0.2 * dec_b[192:256] + 0.5      # ts0 dec odd o
    aps[64:128, 13] = 0.2 * dec_b[0:64] + 0.5       # ts0 dec odd i
    bout = np.tile(np.asarray(inputs["out_b"], np.float32), T).reshape(30, 1)
    # odd-chunk swapped gate layouts for the ts0 cc pair-packing
    encxw0o = np.concatenate([encxw0[:, 64:128], encxw0[:, 0:64]], axis=1)
    decw0o = np.concatenate([decw0[:, :, 64:128], decw0[:, :, 0:64]], axis=2)

    shared = {
        "encw": encw.reshape(128, 5 * 256).astype(BFP),
        "decw0": decw0.reshape(128, 5 * 256).astype(BFP),
        "encxw0": encxw0.astype(BFP),
        "encxw0o": encxw0o.astype(BFP),
        "decw0o": decw0o.reshape(128, 5 * 128).astype(BFP),
        "decw": decw.reshape(128, 9 * 256).astype(BFP),
        "w3": w3.reshape(128, 45 * 30).astype(BFP),
        "aps": aps,
        "bout": bout,
    }
    return [dict(shared, xim=np.ascontiguousarray(xim[c])) for c in range(8)]


_CACHE = {}


def kernel(**inputs):
    if "nc" not in _CACHE:
        _CACHE["nc"] = _build()
    nc = _CACHE["nc"]
    in_maps = _prep(inputs)
    from concourse.bass_utils import run_bass_kernel_spmd
    res = run_bass_kernel_spmd(nc, in_maps, core_ids=list(range(8)))
    kernel.last_exec_ns = res.exec_time_ns
    y = np.stack([
        np.asarray(res.results[c]["y"], np.float32)
        .reshape(T, 3, H, W).transpose(0, 2, 3, 1)
        for c in range(8)
    ])
    return y


# revision 9
# speedup vs baseline: 1.0164x; 1.0164x over previous
"""PredRNN (ConvLSTM enc -> BN -> ConvLSTM dec -> BN -> Conv3D -> sigmoid) on 8 trn2 cores.

Sharding: data-parallel over batch (B=8), one sample per core. Per core:
channel-partition layout, 3x3 convs as shift-window matmuls from padded
bf16 image buffers. Encoder h is stored twice per parity tile ([h; h up1]
in partition halves) so ky-taps pair into K=128 matmuls. Enc t>0 runs at
the 5-pass floor per PSUM half: 3 direct pairs (ky0+ky1 taps), one staged
pair (ky2 kx0+kx1 via column-shifted Pool copies), and one staged K=91
matmul merging the ky2 kx2 tap with the 27-row x-conv (x DMA'd straight
into the staging tile). Staging copies are emitted 2 chunks ahead (and
pre-staged across timestep boundaries) so the Pool queue stays in front
of the PE. Decoder uses K=128 fused [e|h_dec] taps (9/chunk, optimal).
PE pass count 9888 = the bf16 structural floor for this layout.
enc(b) and dec(b-2) are interleaved at every timestep: the enc phase
alone is Act-bound (2448 vs 2133 ns/chunk) while dec has PE slack, so
pairing keeps the whole timestep PE-bound; at ts9 the last chunks are
de-interleaved so the dec-only tail drains the eviction backlog before
conv3d WARs on all 8 PSUM banks. ts0 skips all zero-state h convolutions
(enc: x-conv only; dec: 5-pass e-only conv via decw0: 3 pairs + staged
pair + direct ky2 single) and uses a ts0-only gate packing [i|o],[cc|-]
(f unused) so both relu gates evict in one act op. Gates evict from PSUM
with fused hard-sigmoid, all temps bf16, cell state bf16 in SBUF.
DMA-issue placement matters (Pool SWDGE issues cost ~1us of engine time):
dstage writes + border zeroing + D01 prefetch ride SP/HWDGE, weight loads
avoid the Act queue, dstage border zeroing is spread over t=1..8, and the
D01 prefetch is split into row-bands so no 6us transfer monopolizes the
DMA engines. State-image border memsets are deferred into the ts0 loop
except A0 row 0 / B0 (read early or data-hazardous). Conv3D reuses the
dead state buffers via tag-reuse (A1/B0 prefetched during ts9, A0/B1/ct
after), runs q-outer over 8-block PSUM groups on the pA/pB bank
rotations; sigmoid eviction. Final-timestep state writes are skipped.
t_if (the sole pA reader, heading every eviction chain) rotates 3 deep
so PSUM bank recycling decouples from the DVE chain at phase drains.
ts0 is elementwise-bound (no h-convs), so it runs PAIRED: the cc gates of
chunks (2p, 2p+1) share one PSUM bank at opposite partition halves (odd
chunks use swapped [o|i] weight/bias layouts: encxw0o/decw0o/aps cols
12-13) so a single tanh act evicts both, and the ts0-only e-up image copy
is a Pool tensor_copy of the just-written e rows instead of a second DVE
affine - balancing Act/DVE/Pool just above the PE pace.
"""
import sys

sys.path.insert(0, "/opt/trn_rl_repo")
import numpy as np
import ml_dtypes

import concourse.bass as bass
import concourse.tile as tile
from concourse import mybir
from concourse.vector_clock import ScopedClock

BF16 = mybir.dt.bfloat16
F32 = mybir.dt.float32
AF = mybir.ActivationFunctionType
ALU = mybir.AluOpType

T, H, W, F, C = 10, 128, 128, 64, 3
PW = H + 2
NBLK = H // 4
NPIX = H * W
BN_EPS = 1e-3
BFP = ml_dtypes.bfloat16


def _patched_drain_and_barrier(self, tick_clock, wait_clock):
    nc = self.nc
    carrier = nc.sync.nop(nofuse=True, hint="drain_waits")
    wait_clock.add_sem_waits(carrier.ins, ScopedClock({None: tick_clock.global_clock}))
    si = carrier.ins.sync_info
    waits = list(si.on_wait) if si is not None else []
    if len(waits) > 1:
        si.on_wait = waits[:1]
        for w in waits[1:]:
            n = nc.sync.nop(nofuse=True, hint="drain_waits")
            n.ins.sync_info = mybir.SyncInfo(on_wait=[w], on_update=[])
    nc.sync.drain()
    nc.all_engine_barrier()
    popped = nc._tile_sem_poison_stack.pop()
    assert popped is self._sem_poison
    nc.clear_and_free_semaphores(list(self.sems.allocated().values()))
    nc.all_engine_barrier()


tile.TileContext._drain_and_barrier = _patched_drain_and_barrier


def split_multi_waits(nc, max_keep=1):
    """Walrus codegen rejects >1 sem wait on compute instructions; hoist
    extras onto same-engine single-wait NOPs inserted just before."""
    n_split = 0
    for fn in nc.m.functions:
        for blk in fn.blocks:
            insts = blk.instructions
            i = 0
            while i < len(insts):
                inst = insts[i]
                si = inst.sync_info
                waits = list(si.on_wait) if si is not None and si.on_wait else []
                if len(waits) > max_keep:
                    for j, w in enumerate(waits[:-max_keep]):
                        nop = mybir.InstNoOp(
                            name=f"{inst.name}_w{j}",
                            engine=inst.engine,
                            sync_info=mybir.SyncInfo(on_wait=[w], on_update=[]),
                            bass_nofuse=True,
                            ins=[],
                            outs=[],
                        )
                        insts.insert(i, nop)
                        i += 1
                    si.on_wait = waits[-max_keep:]
                    n_split += 1
                i += 1
    return n_split


def _build(do_split=True):
    nc = bass.Bass()
    xim_d = nc.dram_tensor("xim", [T, 27, NPIX], BF16, kind="ExternalInput")
    encw_d = nc.dram_tensor("encw", [128, 5 * 256], BF16, kind="ExternalInput")
    decw0_d = nc.dram_tensor("decw0", [128, 5 * 256], BF16, kind="ExternalInput")
    encxw0_d = nc.dram_tensor("encxw0", [27, 256], BF16, kind="ExternalInput")
    encxw0o_d = nc.dram_tensor("encxw0o", [27, 128], BF16, kind="ExternalInput")
    decw0o_d = nc.dram_tensor("decw0o", [128, 5 * 128], BF16, kind="ExternalInput")
    decw_d = nc.dram_tensor("decw", [128, 9 * 256], BF16, kind="ExternalInput")
    w3_d = nc.dram_tensor("w3", [128, 45 * 30], BF16, kind="ExternalInput")
    aps_d = nc.dram_tensor("aps", [128, 14], F32, kind="ExternalInput")
    bout_d = nc.dram_tensor("bout", [30, 1], F32, kind="ExternalInput")
    dstage = nc.dram_tensor("dstage", [T, F, PW, PW], BF16, kind="Internal")
    y_d = nc.dram_tensor("y", [30, H, W], F32, kind="ExternalOutput")

    with tile.TileContext(nc) as tc:
        with tc.tile_pool(name="wp", bufs=1) as wp:
            encw = wp.tile([128, 5, 256], BF16)
            encxw0 = wp.tile([27, 256], BF16)
            encxw0o = wp.tile([27, 128], BF16)
            decw0o = wp.tile([128, 5, 128], BF16)
            decw = wp.tile([128, 9, 256], BF16)
            decw0 = wp.tile([128, 5, 256], BF16)
            w3t = wp.tile([128, 45, 30], BF16)
            aps = wp.tile([128, 14], F32)
            bout = wp.tile([30, 1], F32)
            zb = wp.tile([64, PW], BF16)
            # ordered by first use: ts0 needs encxw0+aps immediately, decw0 at
            # dec-ts0, encw/decw only from ts1, w3t/bout at conv3d
            nc.sync.dma_start(encxw0[:], encxw0_d[:])
            nc.scalar.dma_start(encxw0o[:], encxw0o_d[:])
            # aps rides the Act queue (it is Act's own first need, ~3us in);
            # decw0 is issued after the first xs loads inside the ts0 loop
            # path (SP queue order: encxw0, xs(0), decw0 via emission below)
            nc.scalar.dma_start(aps[:], aps_d[:])
            nc.gpsimd.dma_start(encw[:], encw_d[:].rearrange("p (s g) -> p s g", s=5))
            nc.gpsimd.dma_start(decw[:], decw_d[:].rearrange("p (s g) -> p s g", s=9))
            nc.gpsimd.dma_start(w3t[:], w3_d[:].rearrange("p (s g) -> p s g", s=45))
            nc.gpsimd.dma_start(bout[:], bout_d[:])
            nc.vector.memset(zb[:], 0.0)

            with tc.tile_pool(name="sp", bufs=1) as sp, \
                 tc.tile_pool(name="tp", bufs=1) as tp, \
                 tc.tile_pool(name="pp", bufs=1, space=bass.MemorySpace.PSUM) as pp:
                # state tiles carry tags so conv3d can reuse their buffers
                # (tag-reuse = same space, deps tracked) once they go dead
                A0 = sp.tile([128, PW, PW], BF16, tag="A0", name="A0")
                A1 = sp.tile([128, PW, PW], BF16, tag="A1", name="A1")
                B0 = sp.tile([128, PW, PW], BF16, tag="B0", name="B0")
                B1 = sp.tile([128, PW, PW], BF16, tag="B1", name="B1")
                # cell state allocated at PW*PW (not NPIX) so its buffer can
                # host a conv3d frame pair later; only [:, :NPIX] is used
                ct = sp.tile([128, PW * PW], BF16, tag="ct", name="ct")
                # ts0 skips all h-reads, so only the never-written borders of
                # the state images need zeroing (pad rows/cols + bottom row
                # 128). B0 first: dec-ts0 reads it within ~10us; the other
                # tiles aren't read before t=1, so their memsets are emitted
                # lazily inside the ts0 loop (see below) to keep the DVE
                # queue clear for the first evictions.
                # on Pool (idle at startup; DVE would delay the first
                # eviction chains). Pool memset runs at full efficiency.
                for im in (B0,):
                    nc.gpsimd.memset(im[:, 0, :], 0.0)
                    nc.gpsimd.memset(im[:, 128:130, :], 0.0)
                    nc.gpsimd.memset(im[:, :, 0], 0.0)
                    nc.gpsimd.memset(im[:, :, PW - 1], 0.0)
                # A0's bottom-half row 0 is DATA (enc(0,b=0)'s rowup copy
                # writes rows 0:4): this memset must precede that write, so
                # it cannot be deferred with the other lazy strips
                nc.gpsimd.memset(A0[:, 0, :], 0.0)
                # ct needs no memset: every element is written at ts0 (c0 = i*cc)
                # before any read. dstage border zeroing is spread over
                # t=1..8 on SP (only needs to precede the conv3d reads).
                Bs = [B0, B1]
                As = [A0, A1]

                def tmp(tag):
                    # t_if heads every eviction chain (sole pA reader): give
                    # it a deeper rotation so its WAR trails 3 chunk-pairs
                    # instead of 2, decoupling PSUM recycling at drains
                    return tp.tile([128, 512], BF16, tag=tag,
                                   bufs=(3 if tag == "t_if" else 2), name=tag)

                def enc_stage(b, t, Aprev):
                    # ky2 taps relocated/column-shifted into K=128-packable
                    # tiles; x rides partitions 64:91 of s2 (K=91 pass).
                    # Aprev bottom half is h stored rowup: window rows
                    # r0+1:r0+5 = tap ky2, col offset = kx.
                    r0, c0 = 4 * b, 512 * b
                    s1 = tp.tile([128, 4, 128], BF16, tag="s1", bufs=4)
                    s2 = tp.tile([128, 4, 128], BF16, tag="s2", bufs=4)
                    nc.gpsimd.tensor_copy(s1[0:64, :, :], Aprev[64:128, r0 + 1:r0 + 5, 0:128])
                    nc.gpsimd.tensor_copy(s1[64:128, :, :], Aprev[64:128, r0 + 1:r0 + 5, 1:129])
                    nc.gpsimd.tensor_copy(s2[0:64, :, :], Aprev[64:128, r0 + 1:r0 + 5, 2:130])
                    nc.sync.dma_start(
                        s2[64:91, :, :],
                        xim_d[t, :, 512 * b:512 * b + 512].rearrange("p (a b) -> p a b", a=4))
                    return s1, s2

                D01 = []
                for t in range(T):
                    Acur, Aprev = As[t % 2], As[(t - 1) % 2]
                    Bcur, Bnext = Bs[t % 2], Bs[(t + 1) % 2]

                    # ---------------- encoder ----------------
                    def enc_block(b, stage=None, t=t, Acur=Acur, Aprev=Aprev, Bcur=Bcur):
                        r0, c0 = 4 * b, 512 * b
                        if t == 0:
                            xs = tp.tile([27, 512], BF16, tag="s1", bufs=4)
                            nc.sync.dma_start(xs[:], xim_d[t, :, c0:c0 + 512])
                        else:
                            s1, s2 = stage
                        pA = pp.tile([128, 512], F32, tag="pA", bufs=4)
                        pB = pp.tile([128, 512], F32, tag="pB", bufs=4)
                        for ch, ps in ((0, pA), (1, pB)):
                            if t > 0:  # h_{-1}=0: ts0 needs only the x conv
                                for kx in range(3):
                                    # K=128 pair: ky=0 (top) + ky=1 (bottom)
                                    nc.tensor.matmul(
                                        ps[:],
                                        encw[:, kx, 128 * ch:128 * ch + 128],
                                        Aprev[:, r0:r0 + 4, kx:kx + 128],
                                        start=(kx == 0), stop=False)
                                # K=128 staged pair: taps (2,0)+(2,1)
                                nc.tensor.matmul(
                                    ps[:], encw[:, 3, 128 * ch:128 * ch + 128],
                                    s1[:, :, :], start=False, stop=False)
                                # K=91: tap (2,2) + 27-row x-conv
                                nc.tensor.matmul(
                                    ps[:], encw[0:91, 4, 128 * ch:128 * ch + 128],
                                    s2[0:91, :, :], start=False, stop=True)
                            else:
                                nc.tensor.matmul(
                                    ps[:], encxw0[:, 128 * ch:128 * ch + 128], xs[:],
                                    start=True, stop=True)
                        t_if, t_tc, t_o = tmp("t_if"), tmp("t_tc"), tmp("t_o")
                        t_s, t_s2, t_th, t_h = tmp("t_s"), tmp("t_s2"), tmp("t_th"), tmp("t_h")
                        cblk = ct[64:128, c0:c0 + 512]
                        if t == 0:
                            # ts0: f unused (c_{-1}=0); gates packed [i|o],[cc|-]
                            # so both relu gates evict in ONE act op
                            nc.scalar.activation(t_if[:], pA[:], AF.Relu, bias=aps[:, 8:9], scale=0.2)
                            nc.scalar.activation(t_tc[0:64, :], pB[0:64, :], AF.Tanh, bias=aps[0:64, 9:10], scale=1.0)
                            nc.vector.scalar_tensor_tensor(cblk, t_if[0:64, :], 1.0, t_tc[0:64, :], ALU.min, ALU.mult)
                            # t_th on partitions 64:128 so the STT inputs
                            # (o-gate at 64:128) share a base partition
                            nc.scalar.activation(t_th[64:128, :], cblk, AF.Tanh)
                            nc.vector.scalar_tensor_tensor(t_h[0:64, :], t_if[64:128, :], 1.0, t_th[64:128, :], ALU.min, ALU.mult)
                        else:
                            nc.scalar.activation(t_if[:], pA[:], AF.Relu, bias=aps[:, 0:1], scale=0.2)
                            nc.scalar.activation(t_tc[0:64, :], pB[0:64, :], AF.Tanh, bias=aps[0:64, 2:3], scale=1.0)
                            nc.scalar.activation(t_o[0:64, :], pB[64:128, :], AF.Relu, bias=aps[64:128, 2:3], scale=0.2)
                            nc.vector.scalar_tensor_tensor(t_s[64:128, :], t_if[0:64, :], 1.0, t_tc[0:64, :], ALU.min, ALU.mult)
                            nc.vector.scalar_tensor_tensor(t_s2[64:128, :], t_if[64:128, :], 1.0, cblk, ALU.min, ALU.mult)
                            nc.vector.tensor_tensor(cblk, t_s[64:128, :], t_s2[64:128, :], ALU.add)
                            nc.scalar.activation(t_th[0:64, :], cblk, AF.Tanh)
                            nc.vector.scalar_tensor_tensor(t_h[0:64, :], t_o[0:64, :], 1.0, t_th[0:64, :], ALU.min, ALU.mult)
                        hr = t_h[0:64, :].rearrange("p (a b) -> p a b", a=4)
                        if t < T - 1:  # ts9's h_enc is never convolved again
                            nc.gpsimd.tensor_copy(Acur[0:64, r0 + 1:r0 + 5, 1:1 + W], hr)
                            nc.gpsimd.tensor_copy(Acur[64:128, r0:r0 + 4, 1:1 + W], hr)
                        nc.vector.tensor_scalar(
                            Bcur[0:64, r0 + 1:r0 + 5, 1:1 + W], hr,
                            aps[0:64, 4:5], aps[0:64, 5:6], ALU.mult, ALU.add)
                        if t == 0:  # also e down1 into h-half for paired dec ts0
                            nc.vector.tensor_scalar(
                                Bcur[64:128, r0:r0 + 4, 1:1 + W], hr,
                                aps[0:64, 4:5], aps[0:64, 5:6], ALU.mult, ALU.add)

                    def dec_stage0(b, Bcur=Bcur):
                        # ts0 e-only conv: stage the (2,0)+(2,1) pair from the
                        # e-up half of B (column shifts baked into the copies)
                        r0 = 4 * b
                        s1 = tp.tile([128, 4, 128], BF16, tag="s2", bufs=4)
                        nc.gpsimd.tensor_copy(s1[0:64, :, :], Bcur[64:128, r0 + 1:r0 + 5, 0:128])
                        nc.gpsimd.tensor_copy(s1[64:128, :, :], Bcur[64:128, r0 + 1:r0 + 5, 1:129])
                        return s1

                    # ---------------- decoder ----------------
                    def dec_block(b, stage=None, t=t, Bcur=Bcur, Bnext=Bnext):
                        r0, c0 = 4 * b, 512 * b
                        pA = pp.tile([128, 512], F32, tag="pA", bufs=4)
                        pB = pp.tile([128, 512], F32, tag="pB", bufs=4)
                        for ch, ps in ((0, pA), (1, pB)):
                            if t == 0:
                                # h_dec_{-1}=0; Bcur holds [e; e up1] -> pair kys
                                for kx in range(3):
                                    nc.tensor.matmul(
                                        ps[:],
                                        decw0[:, kx, 128 * ch:128 * ch + 128],
                                        Bcur[:, r0:r0 + 4, kx:kx + 128],
                                        start=(kx == 0), stop=False)
                                # staged pair (2,0)+(2,1)
                                nc.tensor.matmul(
                                    ps[:], decw0[:, 3, 128 * ch:128 * ch + 128],
                                    stage[:, :, :], start=False, stop=False)
                                # direct K=64 single (2,2)
                                nc.tensor.matmul(
                                    ps[:], decw0[64:128, 4, 128 * ch:128 * ch + 128],
                                    Bcur[64:128, r0 + 1:r0 + 5, 2:130],
                                    start=False, stop=True)
                            else:
                                for s in range(9):
                                    ky, kx = s // 3, s % 3
                                    nc.tensor.matmul(
                                        ps[:],
                                        decw[:, s, 128 * ch:128 * ch + 128],
                                        Bcur[:, r0 + ky:r0 + ky + 4, kx:kx + 128],
                                        start=(s == 0), stop=(s == 8))
                        t_if, t_tc, t_o = tmp("t_if"), tmp("t_tc"), tmp("t_o")
                        t_s, t_s2, t_th, t_h = tmp("t_s"), tmp("t_s2"), tmp("t_th"), tmp("t_h")
                        cblk = ct[0:64, c0:c0 + 512]
                        if t == 0:
                            # ts0: f unused; decw0 gates packed [i|o],[cc|-]
                            # so both relu gates evict in ONE act op
                            nc.scalar.activation(t_if[:], pA[:], AF.Relu, bias=aps[:, 10:11], scale=0.2)
                            nc.scalar.activation(t_tc[0:64, :], pB[0:64, :], AF.Tanh, bias=aps[0:64, 11:12], scale=1.0)
                            nc.vector.scalar_tensor_tensor(cblk, t_if[0:64, :], 1.0, t_tc[0:64, :], ALU.min, ALU.mult)
                            nc.scalar.activation(t_th[64:128, :], cblk, AF.Tanh)
                            nc.vector.scalar_tensor_tensor(t_h[0:64, :], t_if[64:128, :], 1.0, t_th[64:128, :], ALU.min, ALU.mult)
                        else:
                            # chunk A is [f|i] (host-permuted columns)
                            nc.scalar.activation(t_if[:], pA[:], AF.Relu, bias=aps[:, 1:2], scale=0.2)
                            nc.scalar.activation(t_tc[64:128, :], pB[0:64, :], AF.Tanh, bias=aps[0:64, 3:4], scale=1.0)
                            nc.scalar.activation(t_o[0:64, :], pB[64:128, :], AF.Relu, bias=aps[64:128, 3:4], scale=0.2)
                            nc.vector.scalar_tensor_tensor(t_s2[0:64, :], t_if[0:64, :], 1.0, cblk, ALU.min, ALU.mult)
                            nc.vector.scalar_tensor_tensor(t_s[0:64, :], t_if[64:128, :], 1.0, t_tc[64:128, :], ALU.min, ALU.mult)
                            nc.vector.tensor_tensor(cblk, t_s[0:64, :], t_s2[0:64, :], ALU.add)
                            nc.scalar.activation(t_th[0:64, :], cblk, AF.Tanh)
                            nc.vector.scalar_tensor_tensor(t_h[0:64, :], t_o[0:64, :], 1.0, t_th[0:64, :], ALU.min, ALU.mult)
                        hr = t_h[0:64, :].rearrange("p (a b) -> p a b", a=4)
                        if t < T - 1:  # ts9's h_dec feeds no further timestep
                            nc.gpsimd.tensor_copy(Bnext[64:128, r0 + 1:r0 + 5, 1:1 + W], hr)
                        dtmp = tp.tile([64, 512], BF16, tag="dtmp", bufs=2)
                        nc.vector.tensor_scalar(
                            dtmp[:], t_h[0:64, :],
                            aps[0:64, 6:7], aps[0:64, 7:8], ALU.mult, ALU.add)
                        # SP-issued (HWDGE ~650ns) instead of Pool (SWDGE ~1us):
                        # keeps the Pool queue free for staging + h-writes
                        nc.sync.dma_start(
                            dstage[t, :, r0 + 1:r0 + 5, 1:1 + W],
                            dtmp[:].rearrange("p (a b) -> p a b", a=4))

                    # ---------------- ts0 pair blocks ----------------
                    # cc gates of a chunk pair share ONE PSUM bank at opposite
                    # partition halves (odd chunks use swapped [o|i] weights +
                    # bias columns), so a single 612ns tanh evicts both ccs.
                    # Even chunks' c-STT runs on Pool to balance DVE.
                    def load_xs0(p):
                        # prefetched one pair ahead: the SP queue serializes
                        # ~650ns/issue, so just-in-time loads starve the PE
                        # during the first pairs
                        tiles = []
                        for j in (0, 1):
                            c0 = 512 * (2 * p + j)
                            xs = tp.tile([27, 512], BF16, tag="s1", bufs=4)
                            nc.sync.dma_start(xs[:], xim_d[0, :, c0:c0 + 512])
                            tiles.append(xs)
                        return tiles

                    def enc_pair0(p, xsp, Acur=Acur, Bcur=Bcur):
                        pB = pp.tile([128, 512], F32, tag="pB", bufs=4)
                        gates = []
                        for j in (0, 1):
                            b = 2 * p + j
                            c0 = 512 * b
                            xs = xsp[j]
                            pA = pp.tile([128, 512], F32, tag="pA", bufs=4)
                            nc.tensor.matmul(
                                pA[:],
                                encxw0[:, 0:128] if j == 0 else encxw0o[:],
                                xs[:], start=True, stop=True)
                            nc.tensor.matmul(
                                pB[64 * j:64 * j + 64, :], encxw0[:, 128:192],
                                xs[:], start=True, stop=True)
                            t_if = tmp("t_if")
                            nc.scalar.activation(
                                t_if[:], pA[:], AF.Relu,
                                bias=aps[:, 8 + 4 * j:9 + 4 * j], scale=0.2)
                            gates.append(t_if)
                        t_tc = tmp("t_tc")
                        nc.scalar.activation(t_tc[:], pB[:], AF.Tanh,
                                             bias=aps[:, 9:10], scale=1.0)
                        for j in (0, 1):
                            b = 2 * p + j
                            r0, c0 = 4 * b, 512 * b
                            t_if = gates[j]
                            cblk = ct[64:128, c0:c0 + 512]
                            i_sl = slice(64 * j, 64 * j + 64)
                            o_sl = slice(64 - 64 * j, 128 - 64 * j)
                            nc.vector.scalar_tensor_tensor(
                                cblk, t_if[i_sl, :], 1.0, t_tc[i_sl, :],
                                ALU.min, ALU.mult)
                            t_th, t_h = tmp("t_th"), tmp("t_h")
                            nc.scalar.activation(t_th[o_sl, :], cblk, AF.Tanh)
                            nc.vector.scalar_tensor_tensor(
                                t_h[0:64, :], t_if[o_sl, :], 1.0, t_th[o_sl, :],
                                ALU.min, ALU.mult)
                            hr = t_h[0:64, :].rearrange("p (a b) -> p a b", a=4)
                            nc.gpsimd.tensor_copy(Acur[0:64, r0 + 1:r0 + 5, 1:1 + W], hr)
                            nc.gpsimd.tensor_copy(Acur[64:128, r0:r0 + 4, 1:1 + W], hr)
                            nc.vector.tensor_scalar(
                                Bcur[0:64, r0 + 1:r0 + 5, 1:1 + W], hr,
                                aps[0:64, 4:5], aps[0:64, 5:6], ALU.mult, ALU.add)
                            # e-up = copy of the e rows just written (Pool,
                            # not a second DVE affine: DVE is the ts0 binder)
                            nc.gpsimd.tensor_copy(
                                Bcur[64:128, r0:r0 + 4, 1:1 + W],
                                Bcur[0:64, r0 + 1:r0 + 5, 1:1 + W])

                    def dec_pair0(p, st2, Bcur=Bcur, Bnext=Bnext):
                        pB = pp.tile([128, 512], F32, tag="pB", bufs=4)
                        gates = []
                        for j in (0, 1):
                            b = 2 * p + j
                            r0 = 4 * b
                            st = st2[j]
                            pA = pp.tile([128, 512], F32, tag="pA", bufs=4)
                            wA = decw0 if j == 0 else decw0o
                            for kx in range(3):
                                nc.tensor.matmul(
                                    pA[:], wA[:, kx, 0:128],
                                    Bcur[:, r0:r0 + 4, kx:kx + 128],
                                    start=(kx == 0), stop=False)
                                nc.tensor.matmul(
                                    pB[64 * j:64 * j + 64, :], decw0[:, kx, 128:192],
                                    Bcur[:, r0:r0 + 4, kx:kx + 128],
                                    start=(kx == 0), stop=False)
                            nc.tensor.matmul(
                                pA[:], wA[:, 3, 0:128], st[:, :, :],
                                start=False, stop=False)
                            nc.tensor.matmul(
                                pB[64 * j:64 * j + 64, :], decw0[:, 3, 128:192],
                                st[:, :, :], start=False, stop=False)
                            nc.tensor.matmul(
                                pA[:], wA[64:128, 4, 0:128],
                                Bcur[64:128, r0 + 1:r0 + 5, 2:130],
                                start=False, stop=True)
                            nc.tensor.matmul(
                                pB[64 * j:64 * j + 64, :], decw0[64:128, 4, 128:192],
                                Bcur[64:128, r0 + 1:r0 + 5, 2:130],
                                start=False, stop=True)
                            t_if = tmp("t_if")
                            nc.scalar.activation(
                                t_if[:], pA[:], AF.Relu,
                                bias=aps[:, 10 + 3 * j:11 + 3 * j], scale=0.2)
                            gates.append(t_if)
                        t_tc = tmp("t_tc")
                        nc.scalar.activation(t_tc[:], pB[:], AF.Tanh,
                                             bias=aps[:, 11:12], scale=1.0)
                        for j in (0, 1):
                            b = 2 * p + j
                            r0, c0 = 4 * b, 512 * b
                            t_if = gates[j]
                            cblk = ct[0:64, c0:c0 + 512]
                            i_sl = slice(64 * j, 64 * j + 64)
                            o_sl = slice(64 - 64 * j, 128 - 64 * j)
                            nc.vector.scalar_tensor_tensor(
                                cblk, t_if[i_sl, :], 1.0, t_tc[i_sl, :],
                                ALU.min, ALU.mult)
                            t_th, t_h = tmp("t_th"), tmp("t_h")
                            nc.scalar.activation(t_th[o_sl, :], cblk, AF.Tanh)
                            nc.vector.scalar_tensor_tensor(
                                t_h[0:64, :], t_if[o_sl, :], 1.0, t_th[o_sl, :],
                                ALU.min, ALU.mult)
                            hr = t_h[0:64, :].rearrange("p (a b) -> p a b", a=4)
                            nc.gpsimd.tensor_copy(Bnext[64:128, r0 + 1:r0 + 5, 1:1 + W], hr)
                            dtmp = tp.tile([64, 512], BF16, tag="dtmp", bufs=2)
                            nc.vector.tensor_scalar(
                                dtmp[:], t_h[0:64, :],
                                aps[0:64, 6:7], aps[0:64, 7:8], ALU.mult, ALU.add)
                            nc.sync.dma_start(
                                dstage[0, :, r0 + 1:r0 + 5, 1:1 + W],
                                dtmp[:].rearrange("p (a b) -> p a b", a=4))

                    if t == 0:
                        # coarse interleave: dec blocks (PE-heavy) fill the
                        # PE while enc evictions (act-paced) run; offset 4
                        # keeps dec eviction chains from head-blocking the
                        # act queue (dec(k) needs enc(k+1)'s bottom row)
                        dstages = {}
                        # A0/A1/B1 borders aren't read before t=1: emit their
                        # memsets lazily mid-ts0 to keep the DVE queue clear
                        # for the first eviction chains
                        lazy_ms = []
                        for im in (A0, A1, B1):
                            lazy_ms += [
                                lambda im=im: nc.vector.memset(im[:, 128:130, :], 0.0),
                                lambda im=im: nc.vector.memset(im[:, :, 0], 0.0),
                                lambda im=im: nc.vector.memset(im[:, :, PW - 1], 0.0),
                            ]
                            if im is not A0:  # A0 row 0 is set upfront (data hazard)
                                lazy_ms.append(
                                    lambda im=im: nc.vector.memset(im[:, 0, :], 0.0))
                        NP = NBLK // 2
                        xs_pend = {0: load_xs0(0)}
                        for p in range(NP):
                            if p + 1 < NP:
                                xs_pend[p + 1] = load_xs0(p + 1)
                            enc_pair0(p, xs_pend.pop(p))
                            if p == 1:
                                # behind the first three xs pairs in the SP
                                # queue; dec pair 0 needs these only at ~12us
                                nc.sync.dma_start(
                                    decw0[:],
                                    decw0_d[:].rearrange("p (s g) -> p s g", s=5))
                            if p == 2:
                                nc.sync.dma_start(
                                    decw0o[:],
                                    decw0o_d[:].rearrange("p (s g) -> p s g", s=5))
                            if 3 <= p < 3 + len(lazy_ms):
                                lazy_ms[p - 3]()
                            if p >= 1:
                                dstages[2 * p - 1] = dec_stage0(2 * p - 1)
                            if p + 1 < NP:
                                # chunk 2p needs only enc chunk 2p+1 (this
                                # pair): emit before next pair's Pool writes
                                dstages[2 * p] = dec_stage0(2 * p)
                            if p >= 2:
                                k = 2 * (p - 2)
                                dec_pair0(p - 2, [dstages.pop(k), dstages.pop(k + 1)])
                        # tail: stage emissions interleaved with consumers so
                        # slot reuse never head-blocks the Pool queue
                        dstages[NBLK - 2] = dec_stage0(NBLK - 2)
                        dec_pair0(NP - 2, [dstages.pop(NBLK - 4), dstages.pop(NBLK - 3)])
                        dstages[NBLK - 1] = dec_stage0(NBLK - 1)
                        dec_pair0(NP - 1, [dstages.pop(NBLK - 2), dstages.pop(NBLK - 1)])
                        # pre-stage t=1 chunks 0/1 so enc(1,0) starts clean
                        pend = {0: enc_stage(0, 1, Acur), 1: enc_stage(1, 1, Acur)}
                        continue
                    # dstage borders for earlier frames (conv3d needs them;
                    # nothing reads them before ts9): ~1 frame per timestep
                    bframes = [t - 1] if t < 8 else ([7, 8, 9] if t == 8 else [])
                    for bf in bframes:
                        nc.sync.dma_start(dstage[bf, :, 0, :], zb[:])
                        nc.sync.dma_start(dstage[bf, :, PW - 1, :], zb[:])
                        nc.sync.dma_start(dstage[bf, :, :, 0], zb[:])
                        nc.sync.dma_start(dstage[bf, :, :, PW - 1], zb[:])
                    stages = pend
                    if t == 1:
                        # reclaim B0 bottom row 0 (junked by ts0's e-down1);
                        # dec(1) writes B0 rows 1..128 only
                        nc.vector.memset(Bnext[64:128, 0, :], 0.0)
                    if t == T - 1:
                        # A1/B0 are dead through ts9 (final-ts writes
                        # skipped, last reads at ts8): prefetch conv3d
                        # frames 0..3 into their buffers during dec ts9.
                        # Split into row-bands so no single 6us transfer
                        # monopolizes the DMA engines against the small
                        # latency-critical stage/dstage transfers.
                        D01 = [
                            sp.tile([128, PW, PW], BF16, tag="A1", name="Dp0"),
                            sp.tile([128, PW, PW], BF16, tag="B0", name="Dp1"),
                        ]
                        d01_parts = []
                        nband = 4
                        rb = [0, 33, 66, 99, PW]
                        for q in range(2):
                            for hh, h0 in ((0, 0), (64, 1)):
                                for k in range(nband):
                                    d01_parts.append((q, hh, rb[k], rb[k + 1], h0))
                    # interleave enc(b) with dec(b-2): the enc phase alone is
                    # act-bound (2448 > 2133 ns/chunk) while dec has PE slack;
                    # pairing keeps every chunk-pair PE-bound. dec(k) needs
                    # e rows through 4k+4, written by enc(k+1).
                    def dec_k(k, t=t):
                        if t == T - 1 and k < len(d01_parts):
                            q, hh, ra, rz, h0 = d01_parts[k]
                            nc.sync.dma_start(
                                D01[q][hh:hh + 64, ra:rz, :],
                                dstage[2 * q + h0, :, ra:rz, :])
                        dec_block(k)
                    # at t==T-1, de-interleave the last chunks: the dec-only
                    # tail has PE slack (3840 vs 2448 ns/chunk), letting the
                    # Act/DVE eviction backlog drain before conv3d WARs on
                    # all 8 PSUM banks (else ~10us PE stall at conv3d start)
                    ilv_last = NBLK - 2 if t < T - 1 else 28
                    for b in range(NBLK):
                        if b + 2 < NBLK:
                            stages[b + 2] = enc_stage(b + 2, t, Aprev)
                        enc_block(b, stages.pop(b))
                        if 0 <= b - 2 < ilv_last:
                            dec_k(b - 2)
                    for k in range(ilv_last, NBLK):
                        dec_k(k)
                        if t < T - 1:
                            # pre-stage the next timestep's first two chunks
                            # during the dec tail so enc(t+1,0) starts clean
                            pend = pend if k > NBLK - 2 else {}
                            pend[k - (NBLK - 2)] = enc_stage(
                                k - (NBLK - 2), t + 1, Acur)

                # ---------------- conv3d + sigmoid ----------------
                # frames 0..3 prefetched during ts9 (D01); frames 4..9 load
                # into the now-dead A0/B1/ct buffers via tag reuse
                D = D01 + [
                    sp.tile([128, PW, PW], BF16, tag="A0", name="D2"),
                    sp.tile([128, PW, PW], BF16, tag="B1", name="D3"),
                    sp.tile([128, PW, PW], BF16, tag="ct", name="D4"),
                ]
                dma_engs = [nc.sync, nc.scalar, nc.gpsimd]
                for q in range(2, 5):
                    dma_engs[(2 * q) % 3].dma_start(
                        D[q][0:64, :, :], dstage[2 * q, :, :, :])
                    dma_engs[(2 * q + 1) % 3].dma_start(
                        D[q][64:128, :, :], dstage[2 * q + 1, :, :, :])
                # q-outer over 8-block groups: early matmuls need only D0
                # while later D tiles are still in flight; PSUM groups reuse
                # the pA/pB bank rotations (rows 0:30 of each bank)
                for g in range(NBLK // 8):
                    pys = [pp.tile([128, 512], F32,
                                   tag=("pA" if bb % 2 == 0 else "pB"), bufs=4,
                                   name=f"py{bb}") for bb in range(8)]
                    for q in range(5):
                        for bb in range(8):
                            r0 = 4 * (8 * g + bb)
                            for s in range(9):
                                ky, kx = s // 3, s % 3
                                nc.tensor.matmul(
                                    pys[bb][0:30, :], w3t[:, q * 9 + s, :],
                                    D[q][:, r0 + ky:r0 + ky + 4, kx:kx + 128],
                                    start=(q == 0 and s == 0),
                                    stop=(q == 4 and s == 8))
                    for bb in range(8):
                        r0 = 4 * (8 * g + bb)
                        ty = tp.tile([30, 512], F32, tag="ty", bufs=1)
                        nc.scalar.activation(ty[:], pys[bb][0:30, :], AF.Sigmoid,
                                             bias=bout[:], scale=1.0)
                        nc.scalar.dma_start(
                            y_d[:, r0:r0 + 4, :],
                            ty[:].rearrange("p (a b) -> p a b", a=4))

    if do_split:
        split_multi_waits(nc)
    nc.finalize()
    return nc


def _prep(inputs):
    x = np.asarray(inputs["x"], np.float32)
    xpad = np.zeros((8, T, PW, PW, C), np.float32)
    xpad[:, :, 1:1 + H, 1:1 + W, :] = x
    xim = np.empty((8, T, 27, NPIX), BFP)
    for ky in range(3):
        for kx in range(3):
            s = ky * 3 + kx
            v = xpad[:, :, ky:ky + H, kx:kx + W, :]
            xim[:, :, s * 3:s * 3 + 3, :] = (
                v.transpose(0, 1, 4, 2, 3).reshape(8, T, 3, NPIX).astype(BFP))

    enc_Wh = np.asarray(inputs["enc_Wh"], np.float32)
    enc_Wx = np.asarray(inputs["enc_Wx"], np.float32)
    dec_Wx = np.asarray(inputs["dec_Wx"], np.float32)
    dec_Wh = np.asarray(inputs["dec_Wh"], np.float32)
    out_W = np.asarray(inputs["out_W"], np.float32)

    encw = np.zeros((128, 5, 256), np.float32)
    decw = np.zeros((128, 9, 256), np.float32)
    perm = np.concatenate([np.arange(64, 128), np.arange(0, 64), np.arange(128, 256)])
    perm0 = np.concatenate([np.arange(0, 64), np.arange(192, 256),
                            np.arange(128, 192), np.arange(64, 128)])
    for s in range(9):
        ky, kx = s // 3, s % 3
        decw[0:64, s, :] = dec_Wx[ky, kx][:, perm]
        decw[64:128, s, :] = dec_Wh[ky, kx][:, perm]
    decw0 = np.zeros((128, 5, 256), np.float32)
    for kx in range(3):
        # paired matmul: top half = tap ky=0, bottom = tap ky=1 (e up1 copy)
        encw[0:64, kx, :] = enc_Wh[0, kx]
        encw[64:128, kx, :] = enc_Wh[1, kx]
        # ts0 decoder: e-only paired conv, gates packed [i|o|c|junk]
        decw0[0:64, kx, :] = dec_Wx[0, kx][:, perm0]
        decw0[64:128, kx, :] = dec_Wx[1, kx][:, perm0]
    # staged pair (2,0)+(2,1) and merged (2,2)+x slots
    encw[0:64, 3, :] = enc_Wh[2, 0]
    encw[64:128, 3, :] = enc_Wh[2, 1]
    encw[0:64, 4, :] = enc_Wh[2, 2]
    encw[64:91, 4, :] = enc_Wx.reshape(27, 256)
    decw0[0:64, 3, :] = dec_Wx[2, 0][:, perm0]
    decw0[64:128, 3, :] = dec_Wx[2, 1][:, perm0]
    decw0[64:128, 4, :] = dec_Wx[2, 2][:, perm0]
    encxw = enc_Wx.reshape(27, 256)

    w3 = np.zeros((45, 128, 30), np.float32)
    for q in range(5):
        for j in range(2):
            f = 2 * q + j
            for t in range(max(0, f - 1), min(T - 1, f + 1) + 1):
                dt = f - t + 1
                for s in range(9):
                    ky, kx = s // 3, s % 3
                    w3[q * 9 + s, 64 * j:64 * j + 64, 3 * t:3 * t + 3] = out_W[dt, ky, kx]
    w3 = w3.transpose(1, 0, 2)  # [128, 45, 30]

    enc_b = np.asarray(inputs["enc_b"], np.float32)
    dec_b = np.asarray(inputs["dec_b"], np.float32)
    s_e = np.asarray(inputs["enc_gamma"], np.float32) / np.sqrt(
        np.asarray(inputs["enc_var"], np.float32) + BN_EPS)
    t_e = np.asarray(inputs["enc_beta"], np.float32) - np.asarray(inputs["enc_mean"], np.float32) * s_e
    s_d = np.asarray(inputs["dec_gamma"], np.float32) / np.sqrt(
        np.asarray(inputs["dec_var"], np.float32) + BN_EPS)
    t_d = np.asarray(inputs["dec_beta"], np.float32) - np.asarray(inputs["dec_mean"], np.float32) * s_d

    # ts0 encoder x-conv with gate columns [i|o|c|0] (f unused at ts0)
    encxw0 = np.zeros((27, 256), np.float32)
    encxw0[:, 0:64] = encxw[:, 0:64]
    encxw0[:, 64:128] = encxw[:, 192:256]
    encxw0[:, 128:192] = encxw[:, 128:192]

    aps = np.zeros((128, 14), np.float32)
    aps[0:64, 0] = 0.2 * enc_b[0:64] + 0.5          # enc i
    aps[64:128, 0] = 0.2 * enc_b[64:128] + 0.5      # enc f
    aps[0:64, 1] = 0.2 * dec_b[64:128] + 0.5        # dec f (chunk A is [f|i])
    aps[64:128, 1] = 0.2 * dec_b[0:64] + 0.5        # dec i
    aps[0:64, 2] = enc_b[128:192]                   # enc c~ (tanh bias)
    aps[64:128, 2] = 0.2 * enc_b[192:256] + 0.5     # enc o
    aps[0:64, 3] = dec_b[128:192]
    aps[64:128, 3] = 0.2 * dec_b[192:256] + 0.5
    aps[0:64, 4] = s_e
    aps[0:64, 5] = t_e
    aps[0:64, 6] = s_d
    aps[0:64, 7] = t_d
    aps[0:64, 8] = 0.2 * enc_b[0:64] + 0.5          # ts0 enc i
    aps[64:128, 8] = 0.2 * enc_b[192:256] + 0.5     # ts0 enc o
    aps[0:64, 9] = enc_b[128:192]                   # ts0 enc c~ (tanh bias)
    aps[0:64, 10] = 0.2 * dec_b[0:64] + 0.5         # ts0 dec i
    aps[64:128, 10] = 0.2 * dec_b[192:256] + 0.5    # ts0 dec o
    aps[0:64, 11] = dec_b[128:192]                  # ts0 dec c~ (tanh bias)
    # ts0 pair-packing: cc tanh acts cover both partition halves (odd chunk's
    # cc lands at 64:128), and odd chunks use swapped [o|i] gate layouts
    aps[64:128, 9] = enc_b[128:192]                 # ts0 enc c~ odd half
    aps[64:128, 11] = dec_b[128:192]                # ts0 dec c~ odd half
    aps[0:64, 12] = 0.2 * enc_b[192:256] + 0.5      # ts0 enc odd o
    aps[64:128, 12] = 0.2 * enc_b[0:64] + 0.5       # ts0 enc odd i
    aps[0:64, 13] = 0.2 * dec_b[192:256] + 0.5      # ts0 dec odd o
    aps[64:128, 13] = 0.2 * dec_b[0:64] + 0.5       # ts0 dec odd i
    bout = np.tile(np.asarray(inputs["out_b"], np.float32), T).reshape(30, 1)
    # odd-chunk swapped gate layouts for the ts0 cc pair-packing
    encxw0o = np.concatenate([encxw0[:, 64:128], encxw0[:, 0:64]], axis=1)
    decw0o = np.concatenate([decw0[:, :, 64:128], decw0[:, :, 0:64]], axis=2)

    shared = {
        "encw": encw.reshape(128, 5 * 256).astype(BFP),
        "decw0": decw0.reshape(128, 5 * 256).astype(BFP),
        "encxw0": encxw0.astype(BFP),
        "encxw0o": encxw0o.astype(BFP),
        "decw0o": decw0o.reshape(128, 5 * 128).astype(BFP),
        "decw": decw.reshape(128, 9 * 256).astype(BFP),
        "w3": w3.reshape(128, 45 * 30).astype(BFP),
        "aps": aps,
        "bout": bout,
    }
    return [dict(shared, xim=np.ascontiguousarray(xim[c])) for c in range(8)]


_CACHE = {}


def kernel(**inputs):
    if "nc" not in _CACHE:
        _CACHE["nc"] = _build()
    nc = _CACHE["nc"]
    in_maps = _prep(inputs)
    from concourse.bass_utils import run_bass_kernel_spmd
    res = run_bass_kernel_spmd(nc, in_maps, core_ids=list(range(8)))
    kernel.last_exec_ns = res.exec_time_ns
    y = np.stack([
        np.asarray(res.results[c]["y"], np.float32)
        .reshape(T, 3, H, W).transpose(0, 2, 3, 1)
        for c in range(8)
    ])
    return y


# revision 10
# speedup vs baseline: 1.0174x; 1.0010x over previous
"""PredRNN (ConvLSTM enc -> BN -> ConvLSTM dec -> BN -> Conv3D -> sigmoid) on 8 trn2 cores.

Sharding: data-parallel over batch (B=8), one sample per core. Per core:
channel-partition layout, 3x3 convs as shift-window matmuls from padded
bf16 image buffers. Encoder h is stored twice per parity tile ([h; h up1]
in partition halves) so ky-taps pair into K=128 matmuls. Enc t>0 runs at
the 5-pass floor per PSUM half: 3 direct pairs (ky0+ky1 taps), one staged
pair (ky2 kx0+kx1 via column-shifted Pool copies), and one staged K=91
matmul merging the ky2 kx2 tap with the 27-row x-conv (x DMA'd straight
into the staging tile). Staging copies are emitted 2 chunks ahead (and
pre-staged across timestep boundaries) so the Pool queue stays in front
of the PE. Decoder uses K=128 fused [e|h_dec] taps (9/chunk, optimal).
PE pass count 9888 = the bf16 structural floor for this layout.
enc(b) and dec(b-2) are interleaved at every timestep: the enc phase
alone is Act-bound (2448 vs 2133 ns/chunk) while dec has PE slack, so
pairing keeps the whole timestep PE-bound; at ts9 the last chunks are
de-interleaved so the dec-only tail drains the eviction backlog before
conv3d WARs on all 8 PSUM banks. ts0 skips all zero-state h convolutions
(enc: x-conv only; dec: 5-pass e-only conv via decw0: 3 pairs + staged
pair + direct ky2 single) and uses a ts0-only gate packing [i|o],[cc|-]
(f unused) so both relu gates evict in one act op. Gates evict from PSUM
with fused hard-sigmoid, all temps bf16, cell state bf16 in SBUF.
DMA-issue placement matters (Pool SWDGE issues cost ~1us of engine time):
dstage writes + border zeroing + D01 prefetch ride SP/HWDGE, weight loads
avoid the Act queue, dstage border zeroing is spread over t=1..8, and the
D01 prefetch is split into row-bands so no 6us transfer monopolizes the
DMA engines. State-image border memsets are deferred into the ts0 loop
except A0 row 0 / B0 (read early or data-hazardous). Conv3D reuses the
dead state buffers via tag-reuse (A1/B0 prefetched during ts9, A0/B1/ct
after), runs q-outer over 8-block PSUM groups on the pA/pB bank
rotations; sigmoid eviction. Final-timestep state writes are skipped.
t_if (the sole pA reader, heading every eviction chain) rotates 3 deep
so PSUM bank recycling decouples from the DVE chain at phase drains.
ts0 is elementwise-bound (no h-convs), so it runs PAIRED: the cc gates of
chunks (2p, 2p+1) share one PSUM bank at opposite partition halves (odd
chunks use swapped [o|i] weight/bias layouts: encxw0o/decw0o/aps cols
12-13) so a single tanh act evicts both, and the ts0-only e-up image copy
is a Pool tensor_copy of the just-written e rows instead of a second DVE
affine - balancing Act/DVE/Pool just above the PE pace.
"""
import sys

sys.path.insert(0, "/opt/trn_rl_repo")
import numpy as np
import ml_dtypes

import concourse.bass as bass
import concourse.tile as tile
from concourse import mybir
from concourse.vector_clock import ScopedClock

BF16 = mybir.dt.bfloat16
F32 = mybir.dt.float32
AF = mybir.ActivationFunctionType
ALU = mybir.AluOpType

T, H, W, F, C = 10, 128, 128, 64, 3
PW = H + 2
NBLK = H // 4
NPIX = H * W
BN_EPS = 1e-3
BFP = ml_dtypes.bfloat16


def _patched_drain_and_barrier(self, tick_clock, wait_clock):
    nc = self.nc
    carrier = nc.sync.nop(nofuse=True, hint="drain_waits")
    wait_clock.add_sem_waits(carrier.ins, ScopedClock({None: tick_clock.global_clock}))
    si = carrier.ins.sync_info
    waits = list(si.on_wait) if si is not None else []
    if len(waits) > 1:
        si.on_wait = waits[:1]
        for w in waits[1:]:
            n = nc.sync.nop(nofuse=True, hint="drain_waits")
            n.ins.sync_info = mybir.SyncInfo(on_wait=[w], on_update=[])
    nc.sync.drain()
    nc.all_engine_barrier()
    popped = nc._tile_sem_poison_stack.pop()
    assert popped is self._sem_poison
    nc.clear_and_free_semaphores(list(self.sems.allocated().values()))
    nc.all_engine_barrier()


tile.TileContext._drain_and_barrier = _patched_drain_and_barrier


def split_multi_waits(nc, max_keep=1):
    """Walrus codegen rejects >1 sem wait on compute instructions; hoist
    extras onto same-engine single-wait NOPs inserted just before."""
    n_split = 0
    for fn in nc.m.functions:
        for blk in fn.blocks:
            insts = blk.instructions
            i = 0
            while i < len(insts):
                inst = insts[i]
                si = inst.sync_info
                waits = list(si.on_wait) if si is not None and si.on_wait else []
                if len(waits) > max_keep:
                    for j, w in enumerate(waits[:-max_keep]):
                        nop = mybir.InstNoOp(
                            name=f"{inst.name}_w{j}",
                            engine=inst.engine,
                            sync_info=mybir.SyncInfo(on_wait=[w], on_update=[]),
                            bass_nofuse=True,
                            ins=[],
                            outs=[],
                        )
                        insts.insert(i, nop)
                        i += 1
                    si.on_wait = waits[-max_keep:]
                    n_split += 1
                i += 1
    return n_split


def _build(do_split=True):
    nc = bass.Bass()
    xim_d = nc.dram_tensor("xim", [T, 27, NPIX], BF16, kind="ExternalInput")
    encw_d = nc.dram_tensor("encw", [128, 5 * 256], BF16, kind="ExternalInput")
    decw0_d = nc.dram_tensor("decw0", [128, 5 * 256], BF16, kind="ExternalInput")
    encxw0_d = nc.dram_tensor("encxw0", [27, 256], BF16, kind="ExternalInput")
    encxw0o_d = nc.dram_tensor("encxw0o", [27, 128], BF16, kind="ExternalInput")
    decw0o_d = nc.dram_tensor("decw0o", [128, 5 * 128], BF16, kind="ExternalInput")
    decw_d = nc.dram_tensor("decw", [128, 9 * 256], BF16, kind="ExternalInput")
    w3_d = nc.dram_tensor("w3", [128, 45 * 30], BF16, kind="ExternalInput")
    aps_d = nc.dram_tensor("aps", [128, 14], F32, kind="ExternalInput")
    bout_d = nc.dram_tensor("bout", [30, 1], F32, kind="ExternalInput")
    dstage = nc.dram_tensor("dstage", [T, F, PW, PW], BF16, kind="Internal")
    y_d = nc.dram_tensor("y", [30, H, W], F32, kind="ExternalOutput")

    with tile.TileContext(nc) as tc:
        with tc.tile_pool(name="wp", bufs=1) as wp:
            encw = wp.tile([128, 5, 256], BF16)
            encxw0 = wp.tile([27, 256], BF16)
            encxw0o = wp.tile([27, 128], BF16)
            decw0o = wp.tile([128, 5, 128], BF16)
            decw = wp.tile([128, 9, 256], BF16)
            decw0 = wp.tile([128, 5, 256], BF16)
            w3t = wp.tile([128, 45, 30], BF16)
            aps = wp.tile([128, 14], F32)
            bout = wp.tile([30, 1], F32)
            zb = wp.tile([64, PW], BF16)
            # ordered by first use: ts0 needs encxw0+aps immediately, decw0 at
            # dec-ts0, encw/decw only from ts1, w3t/bout at conv3d
            nc.sync.dma_start(encxw0[:], encxw0_d[:])
            nc.scalar.dma_start(encxw0o[:], encxw0o_d[:])
            # aps rides the Act queue (it is Act's own first need, ~3us in);
            # decw0 is issued after the first xs loads inside the ts0 loop
            # path (SP queue order: encxw0, xs(0), decw0 via emission below)
            nc.scalar.dma_start(aps[:], aps_d[:])
            nc.gpsimd.dma_start(encw[:], encw_d[:].rearrange("p (s g) -> p s g", s=5))
            nc.gpsimd.dma_start(decw[:], decw_d[:].rearrange("p (s g) -> p s g", s=9))
            nc.gpsimd.dma_start(w3t[:], w3_d[:].rearrange("p (s g) -> p s g", s=45))
            nc.gpsimd.dma_start(bout[:], bout_d[:])
            nc.vector.memset(zb[:], 0.0)

            with tc.tile_pool(name="sp", bufs=1) as sp, \
                 tc.tile_pool(name="tp", bufs=1) as tp, \
                 tc.tile_pool(name="pp", bufs=1, space=bass.MemorySpace.PSUM) as pp:
                # state tiles carry tags so conv3d can reuse their buffers
                # (tag-reuse = same space, deps tracked) once they go dead
                A0 = sp.tile([128, PW, PW], BF16, tag="A0", name="A0")
                A1 = sp.tile([128, PW, PW], BF16, tag="A1", name="A1")
                B0 = sp.tile([128, PW, PW], BF16, tag="B0", name="B0")
                B1 = sp.tile([128, PW, PW], BF16, tag="B1", name="B1")
                # cell state allocated at PW*PW (not NPIX) so its buffer can
                # host a conv3d frame pair later; only [:, :NPIX] is used
                ct = sp.tile([128, PW * PW], BF16, tag="ct", name="ct")
                # ts0 skips all h-reads, so only the never-written borders of
                # the state images need zeroing (pad rows/cols + bottom row
                # 128). B0 first: dec-ts0 reads it within ~10us; the other
                # tiles aren't read before t=1, so their memsets are emitted
                # lazily inside the ts0 loop (see below) to keep the DVE
                # queue clear for the first evictions.
                # on Pool (idle at startup; DVE would delay the first
                # eviction chains). Pool memset runs at full efficiency.
                for im in (B0,):
                    nc.gpsimd.memset(im[:, 0, :], 0.0)
                    nc.gpsimd.memset(im[:, 128:130, :], 0.0)
                    nc.gpsimd.memset(im[:, :, 0], 0.0)
                    nc.gpsimd.memset(im[:, :, PW - 1], 0.0)
                # A0's bottom-half row 0 is DATA (enc(0,b=0)'s rowup copy
                # writes rows 0:4): this memset must precede that write, so
                # it cannot be deferred with the other lazy strips
                nc.gpsimd.memset(A0[:, 0, :], 0.0)
                # ct needs no memset: every element is written at ts0 (c0 = i*cc)
                # before any read. dstage border zeroing is spread over
                # t=1..8 on SP (only needs to precede the conv3d reads).
                Bs = [B0, B1]
                As = [A0, A1]

                def tmp(tag):
                    # t_if heads every eviction chain (sole pA reader): give
                    # it a deeper rotation so its WAR trails 3 chunk-pairs
                    # instead of 2, decoupling PSUM recycling at drains
                    return tp.tile([128, 512], BF16, tag=tag,
                                   bufs=(3 if tag == "t_if" else 2), name=tag)

                def enc_stage(b, t, Aprev):
                    # ky2 taps relocated/column-shifted into K=128-packable
                    # tiles; x rides partitions 64:91 of s2 (K=91 pass).
                    # Aprev bottom half is h stored rowup: window rows
                    # r0+1:r0+5 = tap ky2, col offset = kx.
                    r0, c0 = 4 * b, 512 * b
                    s1 = tp.tile([128, 4, 128], BF16, tag="s1", bufs=4)
                    s2 = tp.tile([128, 4, 128], BF16, tag="s2", bufs=4)
                    nc.gpsimd.tensor_copy(s1[0:64, :, :], Aprev[64:128, r0 + 1:r0 + 5, 0:128])
                    nc.gpsimd.tensor_copy(s1[64:128, :, :], Aprev[64:128, r0 + 1:r0 + 5, 1:129])
                    nc.gpsimd.tensor_copy(s2[0:64, :, :], Aprev[64:128, r0 + 1:r0 + 5, 2:130])
                    nc.sync.dma_start(
                        s2[64:91, :, :],
                        xim_d[t, :, 512 * b:512 * b + 512].rearrange("p (a b) -> p a b", a=4))
                    return s1, s2

                D01 = []
                for t in range(T):
                    Acur, Aprev = As[t % 2], As[(t - 1) % 2]
                    Bcur, Bnext = Bs[t % 2], Bs[(t + 1) % 2]

                    # ---------------- encoder ----------------
                    def enc_block(b, stage=None, t=t, Acur=Acur, Aprev=Aprev, Bcur=Bcur):
                        r0, c0 = 4 * b, 512 * b
                        if t == 0:
                            xs = tp.tile([27, 512], BF16, tag="s1", bufs=4)
                            nc.sync.dma_start(xs[:], xim_d[t, :, c0:c0 + 512])
                        else:
                            s1, s2 = stage
                        pA = pp.tile([128, 512], F32, tag="pA", bufs=4)
                        pB = pp.tile([128, 512], F32, tag="pB", bufs=4)
                        for ch, ps in ((0, pA), (1, pB)):
                            if t > 0:  # h_{-1}=0: ts0 needs only the x conv
                                for kx in range(3):
                                    # K=128 pair: ky=0 (top) + ky=1 (bottom)
                                    nc.tensor.matmul(
                                        ps[:],
                                        encw[:, kx, 128 * ch:128 * ch + 128],
                                        Aprev[:, r0:r0 + 4, kx:kx + 128],
                                        start=(kx == 0), stop=False)
                                # K=128 staged pair: taps (2,0)+(2,1)
                                nc.tensor.matmul(
                                    ps[:], encw[:, 3, 128 * ch:128 * ch + 128],
                                    s1[:, :, :], start=False, stop=False)
                                # K=91: tap (2,2) + 27-row x-conv
                                nc.tensor.matmul(
                                    ps[:], encw[0:91, 4, 128 * ch:128 * ch + 128],
                                    s2[0:91, :, :], start=False, stop=True)
                            else:
                                nc.tensor.matmul(
                                    ps[:], encxw0[:, 128 * ch:128 * ch + 128], xs[:],
                                    start=True, stop=True)
                        t_if, t_tc, t_o = tmp("t_if"), tmp("t_tc"), tmp("t_o")
                        t_s, t_s2, t_th, t_h = tmp("t_s"), tmp("t_s2"), tmp("t_th"), tmp("t_h")
                        cblk = ct[64:128, c0:c0 + 512]
                        if t == 0:
                            # ts0: f unused (c_{-1}=0); gates packed [i|o],[cc|-]
                            # so both relu gates evict in ONE act op
                            nc.scalar.activation(t_if[:], pA[:], AF.Relu, bias=aps[:, 8:9], scale=0.2)
                            nc.scalar.activation(t_tc[0:64, :], pB[0:64, :], AF.Tanh, bias=aps[0:64, 9:10], scale=1.0)
                            nc.vector.scalar_tensor_tensor(cblk, t_if[0:64, :], 1.0, t_tc[0:64, :], ALU.min, ALU.mult)
                            # t_th on partitions 64:128 so the STT inputs
                            # (o-gate at 64:128) share a base partition
                            nc.scalar.activation(t_th[64:128, :], cblk, AF.Tanh)
                            nc.vector.scalar_tensor_tensor(t_h[0:64, :], t_if[64:128, :], 1.0, t_th[64:128, :], ALU.min, ALU.mult)
                        else:
                            nc.scalar.activation(t_if[:], pA[:], AF.Relu, bias=aps[:, 0:1], scale=0.2)
                            nc.scalar.activation(t_tc[0:64, :], pB[0:64, :], AF.Tanh, bias=aps[0:64, 2:3], scale=1.0)
                            nc.scalar.activation(t_o[0:64, :], pB[64:128, :], AF.Relu, bias=aps[64:128, 2:3], scale=0.2)
                            nc.vector.scalar_tensor_tensor(t_s[64:128, :], t_if[0:64, :], 1.0, t_tc[0:64, :], ALU.min, ALU.mult)
                            nc.vector.scalar_tensor_tensor(t_s2[64:128, :], t_if[64:128, :], 1.0, cblk, ALU.min, ALU.mult)
                            nc.vector.tensor_tensor(cblk, t_s[64:128, :], t_s2[64:128, :], ALU.add)
                            nc.scalar.activation(t_th[0:64, :], cblk, AF.Tanh)
                            nc.vector.scalar_tensor_tensor(t_h[0:64, :], t_o[0:64, :], 1.0, t_th[0:64, :], ALU.min, ALU.mult)
                        hr = t_h[0:64, :].rearrange("p (a b) -> p a b", a=4)
                        if t < T - 1:  # ts9's h_enc is never convolved again
                            nc.gpsimd.tensor_copy(Acur[0:64, r0 + 1:r0 + 5, 1:1 + W], hr)
                            nc.gpsimd.tensor_copy(Acur[64:128, r0:r0 + 4, 1:1 + W], hr)
                        nc.vector.tensor_scalar(
                            Bcur[0:64, r0 + 1:r0 + 5, 1:1 + W], hr,
                            aps[0:64, 4:5], aps[0:64, 5:6], ALU.mult, ALU.add)
                        if t == 0:  # also e down1 into h-half for paired dec ts0
                            nc.vector.tensor_scalar(
                                Bcur[64:128, r0:r0 + 4, 1:1 + W], hr,
                                aps[0:64, 4:5], aps[0:64, 5:6], ALU.mult, ALU.add)

                    def dec_stage0(b, Bcur=Bcur):
                        # ts0 e-only conv: stage the (2,0)+(2,1) pair from the
                        # e-up half of B (column shifts baked into the copies)
                        r0 = 4 * b
                        s1 = tp.tile([128, 4, 128], BF16, tag="s2", bufs=4)
                        nc.gpsimd.tensor_copy(s1[0:64, :, :], Bcur[64:128, r0 + 1:r0 + 5, 0:128])
                        nc.gpsimd.tensor_copy(s1[64:128, :, :], Bcur[64:128, r0 + 1:r0 + 5, 1:129])
                        return s1

                    # ---------------- decoder ----------------
                    def dec_block(b, stage=None, t=t, Bcur=Bcur, Bnext=Bnext):
                        r0, c0 = 4 * b, 512 * b
                        pA = pp.tile([128, 512], F32, tag="pA", bufs=4)
                        pB = pp.tile([128, 512], F32, tag="pB", bufs=4)
                        for ch, ps in ((0, pA), (1, pB)):
                            if t == 0:
                                # h_dec_{-1}=0; Bcur holds [e; e up1] -> pair kys
                                for kx in range(3):
                                    nc.tensor.matmul(
                                        ps[:],
                                        decw0[:, kx, 128 * ch:128 * ch + 128],
                                        Bcur[:, r0:r0 + 4, kx:kx + 128],
                                        start=(kx == 0), stop=False)
                                # staged pair (2,0)+(2,1)
                                nc.tensor.matmul(
                                    ps[:], decw0[:, 3, 128 * ch:128 * ch + 128],
                                    stage[:, :, :], start=False, stop=False)
                                # direct K=64 single (2,2)
                                nc.tensor.matmul(
                                    ps[:], decw0[64:128, 4, 128 * ch:128 * ch + 128],
                                    Bcur[64:128, r0 + 1:r0 + 5, 2:130],
                                    start=False, stop=True)
                            else:
                                for s in range(9):
                                    ky, kx = s // 3, s % 3
                                    nc.tensor.matmul(
                                        ps[:],
                                        decw[:, s, 128 * ch:128 * ch + 128],
                                        Bcur[:, r0 + ky:r0 + ky + 4, kx:kx + 128],
                                        start=(s == 0), stop=(s == 8))
                        t_if, t_tc, t_o = tmp("t_if"), tmp("t_tc"), tmp("t_o")
                        t_s, t_s2, t_th, t_h = tmp("t_s"), tmp("t_s2"), tmp("t_th"), tmp("t_h")
                        cblk = ct[0:64, c0:c0 + 512]
                        if t == 0:
                            # ts0: f unused; decw0 gates packed [i|o],[cc|-]
                            # so both relu gates evict in ONE act op
                            nc.scalar.activation(t_if[:], pA[:], AF.Relu, bias=aps[:, 10:11], scale=0.2)
                            nc.scalar.activation(t_tc[0:64, :], pB[0:64, :], AF.Tanh, bias=aps[0:64, 11:12], scale=1.0)
                            nc.vector.scalar_tensor_tensor(cblk, t_if[0:64, :], 1.0, t_tc[0:64, :], ALU.min, ALU.mult)
                            nc.scalar.activation(t_th[64:128, :], cblk, AF.Tanh)
                            nc.vector.scalar_tensor_tensor(t_h[0:64, :], t_if[64:128, :], 1.0, t_th[64:128, :], ALU.min, ALU.mult)
                        else:
                            # chunk A is [f|i] (host-permuted columns)
                            nc.scalar.activation(t_if[:], pA[:], AF.Relu, bias=aps[:, 1:2], scale=0.2)
                            nc.scalar.activation(t_tc[64:128, :], pB[0:64, :], AF.Tanh, bias=aps[0:64, 3:4], scale=1.0)
                            nc.scalar.activation(t_o[0:64, :], pB[64:128, :], AF.Relu, bias=aps[64:128, 3:4], scale=0.2)
                            nc.vector.scalar_tensor_tensor(t_s2[0:64, :], t_if[0:64, :], 1.0, cblk, ALU.min, ALU.mult)
                            nc.vector.scalar_tensor_tensor(t_s[0:64, :], t_if[64:128, :], 1.0, t_tc[64:128, :], ALU.min, ALU.mult)
                            nc.vector.tensor_tensor(cblk, t_s[0:64, :], t_s2[0:64, :], ALU.add)
                            nc.scalar.activation(t_th[0:64, :], cblk, AF.Tanh)
                            nc.vector.scalar_tensor_tensor(t_h[0:64, :], t_o[0:64, :], 1.0, t_th[0:64, :], ALU.min, ALU.mult)
                        hr = t_h[0:64, :].rearrange("p (a b) -> p a b", a=4)
                        if t < T - 1:  # ts9's h_dec feeds no further timestep
                            nc.gpsimd.tensor_copy(Bnext[64:128, r0 + 1:r0 + 5, 1:1 + W], hr)
                        dtmp = tp.tile([64, 512], BF16, tag="dtmp", bufs=2)
                        nc.vector.tensor_scalar(
                            dtmp[:], t_h[0:64, :],
                            aps[0:64, 6:7], aps[0:64, 7:8], ALU.mult, ALU.add)
                        # SP-issued (HWDGE ~650ns) instead of Pool (SWDGE ~1us):
                        # keeps the Pool queue free for staging + h-writes
                        nc.sync.dma_start(
                            dstage[t, :, r0 + 1:r0 + 5, 1:1 + W],
                            dtmp[:].rearrange("p (a b) -> p a b", a=4))

                    # ---------------- ts0 pair blocks ----------------
                    # cc gates of a chunk pair share ONE PSUM bank at opposite
                    # partition halves (odd chunks use swapped [o|i] weights +
                    # bias columns), so a single 612ns tanh evicts both ccs.
                    # Even chunks' c-STT runs on Pool to balance DVE.
                    def load_xs0(p):
                        # prefetched one pair ahead: the SP queue serializes
                        # ~650ns/issue, so just-in-time loads starve the PE
                        # during the first pairs
                        tiles = []
                        for j in (0, 1):
                            c0 = 512 * (2 * p + j)
                            xs = tp.tile([27, 512], BF16, tag="s1", bufs=4)
                            nc.sync.dma_start(xs[:], xim_d[0, :, c0:c0 + 512])
                            tiles.append(xs)
                        return tiles

                    def enc_pair0(p, xsp, Acur=Acur, Bcur=Bcur):
                        pB = pp.tile([128, 512], F32, tag="pB", bufs=4)
                        gates = []
                        for j in (0, 1):
                            b = 2 * p + j
                            c0 = 512 * b
                            xs = xsp[j]
                            pA = pp.tile([128, 512], F32, tag="pA", bufs=4)
                            nc.tensor.matmul(
                                pA[:],
                                encxw0[:, 0:128] if j == 0 else encxw0o[:],
                                xs[:], start=True, stop=True)
                            nc.tensor.matmul(
                                pB[64 * j:64 * j + 64, :], encxw0[:, 128:192],
                                xs[:], start=True, stop=True)
                            t_if = tmp("t_if")
                            nc.scalar.activation(
                                t_if[:], pA[:], AF.Relu,
                                bias=aps[:, 8 + 4 * j:9 + 4 * j], scale=0.2)
                            gates.append(t_if)
                        t_tc = tmp("t_tc")
                        nc.scalar.activation(t_tc[:], pB[:], AF.Tanh,
                                             bias=aps[:, 9:10], scale=1.0)
                        for j in (0, 1):
                            b = 2 * p + j
                            r0, c0 = 4 * b, 512 * b
                            t_if = gates[j]
                            cblk = ct[64:128, c0:c0 + 512]
                            i_sl = slice(64 * j, 64 * j + 64)
                            o_sl = slice(64 - 64 * j, 128 - 64 * j)
                            nc.vector.scalar_tensor_tensor(
                                cblk, t_if[i_sl, :], 1.0, t_tc[i_sl, :],
                                ALU.min, ALU.mult)
                            t_th, t_h = tmp("t_th"), tmp("t_h")
                            nc.scalar.activation(t_th[o_sl, :], cblk, AF.Tanh)
                            nc.vector.scalar_tensor_tensor(
                                t_h[0:64, :], t_if[o_sl, :], 1.0, t_th[o_sl, :],
                                ALU.min, ALU.mult)
                            hr = t_h[0:64, :].rearrange("p (a b) -> p a b", a=4)
                            nc.gpsimd.tensor_copy(Acur[0:64, r0 + 1:r0 + 5, 1:1 + W], hr)
                            nc.gpsimd.tensor_copy(Acur[64:128, r0:r0 + 4, 1:1 + W], hr)
                            nc.vector.tensor_scalar(
                                Bcur[0:64, r0 + 1:r0 + 5, 1:1 + W], hr,
                                aps[0:64, 4:5], aps[0:64, 5:6], ALU.mult, ALU.add)
                            # e-up = copy of the e rows just written (Pool,
                            # not a second DVE affine: DVE is the ts0 binder)
                            nc.gpsimd.tensor_copy(
                                Bcur[64:128, r0:r0 + 4, 1:1 + W],
                                Bcur[0:64, r0 + 1:r0 + 5, 1:1 + W])

                    def dec_pair0(p, st2, Bcur=Bcur, Bnext=Bnext):
                        pB = pp.tile([128, 512], F32, tag="pB", bufs=4)
                        gates = []
                        for j in (0, 1):
                            b = 2 * p + j
                            r0 = 4 * b
                            st = st2[j]
                            pA = pp.tile([128, 512], F32, tag="pA", bufs=4)
                            wA = decw0 if j == 0 else decw0o
                            for kx in range(3):
                                nc.tensor.matmul(
                                    pA[:], wA[:, kx, 0:128],
                                    Bcur[:, r0:r0 + 4, kx:kx + 128],
                                    start=(kx == 0), stop=False)
                                nc.tensor.matmul(
                                    pB[64 * j:64 * j + 64, :], decw0[:, kx, 128:192],
                                    Bcur[:, r0:r0 + 4, kx:kx + 128],
                                    start=(kx == 0), stop=False)
                            nc.tensor.matmul(
                                pA[:], wA[:, 3, 0:128], st[:, :, :],
                                start=False, stop=False)
                            nc.tensor.matmul(
                                pB[64 * j:64 * j + 64, :], decw0[:, 3, 128:192],
                                st[:, :, :], start=False, stop=False)
                            nc.tensor.matmul(
                                pA[:], wA[64:128, 4, 0:128],
                                Bcur[64:128, r0 + 1:r0 + 5, 2:130],
                                start=False, stop=True)
                            nc.tensor.matmul(
                                pB[64 * j:64 * j + 64, :], decw0[64:128, 4, 128:192],
                                Bcur[64:128, r0 + 1:r0 + 5, 2:130],
                                start=False, stop=True)
                            t_if = tmp("t_if")
                            nc.scalar.activation(
                                t_if[:], pA[:], AF.Relu,
                                bias=aps[:, 10 + 3 * j:11 + 3 * j], scale=0.2)
                            gates.append(t_if)
                        t_tc = tmp("t_tc")
                        nc.scalar.activation(t_tc[:], pB[:], AF.Tanh,
                                             bias=aps[:, 11:12], scale=1.0)
                        for j in (0, 1):
                            b = 2 * p + j
                            r0, c0 = 4 * b, 512 * b
                            t_if = gates[j]
                            cblk = ct[0:64, c0:c0 + 512]
                            i_sl = slice(64 * j, 64 * j + 64)
                            o_sl = slice(64 - 64 * j, 128 - 64 * j)
                            nc.vector.scalar_tensor_tensor(
                                cblk, t_if[i_sl, :], 1.0, t_tc[i_sl, :],
                                ALU.min, ALU.mult)
                            t_th, t_h = tmp("t_th"), tmp("t_h")
                            nc.scalar.activation(t_th[o_sl, :], cblk, AF.Tanh)
                            nc.vector.scalar_tensor_tensor(
                                t_h[0:64, :], t_if[o_sl, :], 1.0, t_th[o_sl, :],
                                ALU.min, ALU.mult)
                            hr = t_h[0:64, :].rearrange("p (a b) -> p a b", a=4)
                            nc.gpsimd.tensor_copy(Bnext[64:128, r0 + 1:r0 + 5, 1:1 + W], hr)
                            dtmp = tp.tile([64, 512], BF16, tag="dtmp", bufs=2)
                            nc.vector.tensor_scalar(
                                dtmp[:], t_h[0:64, :],
                                aps[0:64, 6:7], aps[0:64, 7:8], ALU.mult, ALU.add)
                            nc.sync.dma_start(
                                dstage[0, :, r0 + 1:r0 + 5, 1:1 + W],
                                dtmp[:].rearrange("p (a b) -> p a b", a=4))

                    if t == 0:
                        # coarse interleave: dec blocks (PE-heavy) fill the
                        # PE while enc evictions (act-paced) run; offset 4
                        # keeps dec eviction chains from head-blocking the
                        # act queue (dec(k) needs enc(k+1)'s bottom row)
                        dstages = {}
                        # A0/A1/B1 borders aren't read before t=1: emit their
                        # memsets lazily mid-ts0 to keep the DVE queue clear
                        # for the first eviction chains
                        lazy_ms = []
                        for im in (A0, A1, B1):
                            lazy_ms += [
                                lambda im=im: nc.vector.memset(im[:, 128:130, :], 0.0),
                                lambda im=im: nc.vector.memset(im[:, :, 0], 0.0),
                                lambda im=im: nc.vector.memset(im[:, :, PW - 1], 0.0),
                            ]
                            if im is not A0:  # A0 row 0 is set upfront (data hazard)
                                lazy_ms.append(
                                    lambda im=im: nc.vector.memset(im[:, 0, :], 0.0))
                        NP = NBLK // 2
                        xs_pend = {0: load_xs0(0)}
                        for p in range(NP):
                            if p + 1 < NP:
                                xs_pend[p + 1] = load_xs0(p + 1)
                            enc_pair0(p, xs_pend.pop(p))
                            if p == 1:
                                # behind the first three xs pairs in the SP
                                # queue; dec pair 0 needs these only at ~12us
                                nc.sync.dma_start(
                                    decw0[:],
                                    decw0_d[:].rearrange("p (s g) -> p s g", s=5))
                            if p == 2:
                                nc.sync.dma_start(
                                    decw0o[:],
                                    decw0o_d[:].rearrange("p (s g) -> p s g", s=5))
                            if 3 <= p < 3 + len(lazy_ms):
                                lazy_ms[p - 3]()
                            if p >= 1:
                                dstages[2 * p - 1] = dec_stage0(2 * p - 1)
                            if p + 1 < NP:
                                # chunk 2p needs only enc chunk 2p+1 (this
                                # pair): emit before next pair's Pool writes
                                dstages[2 * p] = dec_stage0(2 * p)
                            if p >= 2:
                                k = 2 * (p - 2)
                                dec_pair0(p - 2, [dstages.pop(k), dstages.pop(k + 1)])
                        # tail: stage emissions interleaved with consumers so
                        # slot reuse never head-blocks the Pool queue
                        dstages[NBLK - 2] = dec_stage0(NBLK - 2)
                        dec_pair0(NP - 2, [dstages.pop(NBLK - 4), dstages.pop(NBLK - 3)])
                        dstages[NBLK - 1] = dec_stage0(NBLK - 1)
                        dec_pair0(NP - 1, [dstages.pop(NBLK - 2), dstages.pop(NBLK - 1)])
                        # cross-boundary pipelining: run enc(1, 0/1) now
                        # (their deps - A0, stages - are ready) so their PSUM
                        # groups allocate before the ts0 eviction backlog
                        # drains; t=1's loop then starts at chunk 2
                        s0 = enc_stage(0, 1, Acur)
                        enc_block(0, s0, t=1, Acur=As[1], Aprev=As[0], Bcur=Bs[1])
                        s1b = enc_stage(1, 1, Acur)
                        enc_block(1, s1b, t=1, Acur=As[1], Aprev=As[0], Bcur=Bs[1])
                        pend = {2: enc_stage(2, 1, Acur), 3: enc_stage(3, 1, Acur)}
                        continue
                    # dstage borders for earlier frames (conv3d needs them;
                    # nothing reads them before ts9): ~1 frame per timestep
                    bframes = [t - 1] if t < 8 else ([7, 8, 9] if t == 8 else [])
                    for bf in bframes:
                        nc.sync.dma_start(dstage[bf, :, 0, :], zb[:])
                        nc.sync.dma_start(dstage[bf, :, PW - 1, :], zb[:])
                        nc.sync.dma_start(dstage[bf, :, :, 0], zb[:])
                        nc.sync.dma_start(dstage[bf, :, :, PW - 1], zb[:])
                    stages = pend
                    if t == 1:
                        # reclaim B0 bottom row 0 (junked by ts0's e-down1);
                        # dec(1) writes B0 rows 1..128 only
                        nc.vector.memset(Bnext[64:128, 0, :], 0.0)
                    if t == T - 1:
                        # A1/B0 are dead through ts9 (final-ts writes
                        # skipped, last reads at ts8): prefetch conv3d
                        # frames 0..3 into their buffers during dec ts9.
                        # Split into row-bands so no single 6us transfer
                        # monopolizes the DMA engines against the small
                        # latency-critical stage/dstage transfers.
                        D01 = [
                            sp.tile([128, PW, PW], BF16, tag="A1", name="Dp0"),
                            sp.tile([128, PW, PW], BF16, tag="B0", name="Dp1"),
                        ]
                        d01_parts = []
                        nband = 4
                        rb = [0, 33, 66, 99, PW]
                        for q in range(2):
                            for hh, h0 in ((0, 0), (64, 1)):
                                for k in range(nband):
                                    d01_parts.append((q, hh, rb[k], rb[k + 1], h0))
                    # interleave enc(b) with dec(b-2): the enc phase alone is
                    # act-bound (2448 > 2133 ns/chunk) while dec has PE slack;
                    # pairing keeps every chunk-pair PE-bound. dec(k) needs
                    # e rows through 4k+4, written by enc(k+1).
                    def dec_k(k, t=t):
                        if t == T - 1 and k < len(d01_parts):
                            q, hh, ra, rz, h0 = d01_parts[k]
                            nc.sync.dma_start(
                                D01[q][hh:hh + 64, ra:rz, :],
                                dstage[2 * q + h0, :, ra:rz, :])
                        dec_block(k)
                    # at t==T-1, de-interleave the last chunks: the dec-only
                    # tail has PE slack (3840 vs 2448 ns/chunk), letting the
                    # Act/DVE eviction backlog drain before conv3d WARs on
                    # all 8 PSUM banks (else ~10us PE stall at conv3d start)
                    ilv_last = NBLK - 2 if t < T - 1 else 28
                    # chunks 0/1 of this timestep already ran during the
                    # previous timestep's dec tail (cross-boundary pipelining)
                    for b in range(2, NBLK):
                        if b + 2 < NBLK:
                            stages[b + 2] = enc_stage(b + 2, t, Aprev)
                        enc_block(b, stages.pop(b))
                        if 0 <= b - 2 < ilv_last:
                            dec_k(b - 2)
                    for k in range(ilv_last, NBLK):
                        dec_k(k)
                        if t < T - 1 and k >= NBLK - 2:
                            j = k - (NBLK - 2)
                            sj = enc_stage(j, t + 1, Acur)
                            enc_block(j, sj, t=t + 1, Acur=As[(t + 1) % 2],
                                      Aprev=As[t % 2], Bcur=Bs[(t + 1) % 2])
                            if k == NBLK - 1:
                                pend = {2: enc_stage(2, t + 1, Acur),
                                        3: enc_stage(3, t + 1, Acur)}

                # ---------------- conv3d + sigmoid ----------------
                # frames 0..3 prefetched during ts9 (D01); frames 4..9 load
                # into the now-dead A0/B1/ct buffers via tag reuse
                D = D01 + [
                    sp.tile([128, PW, PW], BF16, tag="A0", name="D2"),
                    sp.tile([128, PW, PW], BF16, tag="B1", name="D3"),
                    sp.tile([128, PW, PW], BF16, tag="ct", name="D4"),
                ]
                dma_engs = [nc.sync, nc.scalar, nc.gpsimd]
                for q in range(2, 5):
                    dma_engs[(2 * q) % 3].dma_start(
                        D[q][0:64, :, :], dstage[2 * q, :, :, :])
                    dma_engs[(2 * q + 1) % 3].dma_start(
                        D[q][64:128, :, :], dstage[2 * q + 1, :, :, :])
                # q-outer over 8-block groups: early matmuls need only D0
                # while later D tiles are still in flight; PSUM groups reuse
                # the pA/pB bank rotations (rows 0:30 of each bank)
                for g in range(NBLK // 8):
                    pys = [pp.tile([128, 512], F32,
                                   tag=("pA" if bb % 2 == 0 else "pB"), bufs=4,
                                   name=f"py{bb}") for bb in range(8)]
                    for q in range(5):
                        for bb in range(8):
                            r0 = 4 * (8 * g + bb)
                            for s in range(9):
                                ky, kx = s // 3, s % 3
                                nc.tensor.matmul(
                                    pys[bb][0:30, :], w3t[:, q * 9 + s, :],
                                    D[q][:, r0 + ky:r0 + ky + 4, kx:kx + 128],
                                    start=(q == 0 and s == 0),
                                    stop=(q == 4 and s == 8))
                    for bb in range(8):
                        r0 = 4 * (8 * g + bb)
                        ty = tp.tile([30, 512], F32, tag="ty", bufs=1)
                        nc.scalar.activation(ty[:], pys[bb][0:30, :], AF.Sigmoid,
                                             bias=bout[:], scale=1.0)
                        nc.scalar.dma_start(
                            y_d[:, r0:r0 + 4, :],
                            ty[:].rearrange("p (a b) -> p a b", a=4))

    if do_split:
        split_multi_waits(nc)
    nc.finalize()
    return nc


def _prep(inputs):
    x = np.asarray(inputs["x"], np.float32)
    xpad = np.zeros((8, T, PW, PW, C), np.float32)
    xpad[:, :, 1:1 + H, 1:1 + W, :] = x
    xim = np.empty((8, T, 27, NPIX), BFP)
    for ky in range(3):
        for kx in range(3):
            s = ky * 3 + kx
            v = xpad[:, :, ky:ky + H, kx:kx + W, :]
            xim[:, :, s * 3:s * 3 + 3, :] = (
                v.transpose(0, 1, 4, 2, 3).reshape(8, T, 3, NPIX).astype(BFP))

    enc_Wh = np.asarray(inputs["enc_Wh"], np.float32)
    enc_Wx = np.asarray(inputs["enc_Wx"], np.float32)
    dec_Wx = np.asarray(inputs["dec_Wx"], np.float32)
    dec_Wh = np.asarray(inputs["dec_Wh"], np.float32)
    out_W = np.asarray(inputs["out_W"], np.float32)

    encw = np.zeros((128, 5, 256), np.float32)
    decw = np.zeros((128, 9, 256), np.float32)
    perm = np.concatenate([np.arange(64, 128), np.arange(0, 64), np.arange(128, 256)])
    perm0 = np.concatenate([np.arange(0, 64), np.arange(192, 256),
                            np.arange(128, 192), np.arange(64, 128)])
    for s in range(9):
        ky, kx = s // 3, s % 3
        decw[0:64, s, :] = dec_Wx[ky, kx][:, perm]
        decw[64:128, s, :] = dec_Wh[ky, kx][:, perm]
    decw0 = np.zeros((128, 5, 256), np.float32)
    for kx in range(3):
        # paired matmul: top half = tap ky=0, bottom = tap ky=1 (e up1 copy)
        encw[0:64, kx, :] = enc_Wh[0, kx]
        encw[64:128, kx, :] = enc_Wh[1, kx]
        # ts0 decoder: e-only paired conv, gates packed [i|o|c|junk]
        decw0[0:64, kx, :] = dec_Wx[0, kx][:, perm0]
        decw0[64:128, kx, :] = dec_Wx[1, kx][:, perm0]
    # staged pair (2,0)+(2,1) and merged (2,2)+x slots
    encw[0:64, 3, :] = enc_Wh[2, 0]
    encw[64:128, 3, :] = enc_Wh[2, 1]
    encw[0:64, 4, :] = enc_Wh[2, 2]
    encw[64:91, 4, :] = enc_Wx.reshape(27, 256)
    decw0[0:64, 3, :] = dec_Wx[2, 0][:, perm0]
    decw0[64:128, 3, :] = dec_Wx[2, 1][:, perm0]
    decw0[64:128, 4, :] = dec_Wx[2, 2][:, perm0]
    encxw = enc_Wx.reshape(27, 256)

    w3 = np.zeros((45, 128, 30), np.float32)
    for q in range(5):
        for j in range(2):
            f = 2 * q + j
            for t in range(max(0, f - 1), min(T - 1, f + 1) + 1):
                dt = f - t + 1
                for s in range(9):
                    ky, kx = s // 3, s % 3
                    w3[q * 9 + s, 64 * j:64 * j + 64, 3 * t:3 * t + 3] = out_W[dt, ky, kx]
    w3 = w3.transpose(1, 0, 2)  # [128, 45, 30]

    enc_b = np.asarray(inputs["enc_b"], np.float32)
    dec_b = np.asarray(inputs["dec_b"], np.float32)
    s_e = np.asarray(inputs["enc_gamma"], np.float32) / np.sqrt(
        np.asarray(inputs["enc_var"], np.float32) + BN_EPS)
    t_e = np.asarray(inputs["enc_beta"], np.float32) - np.asarray(inputs["enc_mean"], np.float32) * s_e
    s_d = np.asarray(inputs["dec_gamma"], np.float32) / np.sqrt(
        np.asarray(inputs["dec_var"], np.float32) + BN_EPS)
    t_d = np.asarray(inputs["dec_beta"], np.float32) - np.asarray(inputs["dec_mean"], np.float32) * s_d

    # ts0 encoder x-conv with gate columns [i|o|c|0] (f unused at ts0)
    encxw0 = np.zeros((27, 256), np.float32)
    encxw0[:, 0:64] = encxw[:, 0:64]
    encxw0[:, 64:128] = encxw[:, 192:256]
    encxw0[:, 128:192] = encxw[:, 128:192]

    aps = np.zeros((128, 14), np.float32)
    aps[0:64, 0] = 0.2 * enc_b[0:64] + 0.5          # enc i
    aps[64:128, 0] = 0.2 * enc_b[64:128] + 0.5      # enc f
    aps[0:64, 1] = 0.2 * dec_b[64:128] + 0.5        # dec f (chunk A is [f|i])
    aps[64:128, 1] = 0.2 * dec_b[0:64] + 0.5        # dec i
    aps[0:64, 2] = enc_b[128:192]                   # enc c~ (tanh bias)
    aps[64:128, 2] = 0.2 * enc_b[192:256] + 0.5     # enc o
    aps[0:64, 3] = dec_b[128:192]
    aps[64:128, 3] = 0.2 * dec_b[192:256] + 0.5
    aps[0:64, 4] = s_e
    aps[0:64, 5] = t_e
    aps[0:64, 6] = s_d
    aps[0:64, 7] = t_d
    aps[0:64, 8] = 0.2 * enc_b[0:64] + 0.5          # ts0 enc i
    aps[64:128, 8] = 0.2 * enc_b[192:256] + 0.5     # ts0 enc o
    aps[0:64, 9] = enc_b[128:192]                   # ts0 enc c~ (tanh bias)
    aps[0:64, 10] = 0.2 * dec_b[0:64] + 0.5         # ts0 dec i
    aps[64:128, 10] = 0.2 * dec_b[192:256] + 0.5    # ts0 dec o
    aps[0:64, 11] = dec_b[128:192]                  # ts0 dec c~ (tanh bias)
    # ts0 pair-packing: cc tanh acts cover both partition halves (odd chunk's
    # cc lands at 64:128), and odd chunks use swapped [o|i] gate layouts
    aps[64:128, 9] = enc_b[128:192]                 # ts0 enc c~ odd half
    aps[64:128, 11] = dec_b[128:192]                # ts0 dec c~ odd half
    aps[0:64, 12] = 0.2 * enc_b[192:256] + 0.5      # ts0 enc odd o
    aps[64:128, 12] = 0.2 * enc_b[0:64] + 0.5       # ts0 enc odd i
    aps[0:64, 13] = 0.2 * dec_b[192:256] + 0.5      # ts0 dec odd o
    aps[64:128, 13] = 0.2 * dec_b[0:64] + 0.5       # ts0 dec odd i
    bout = np.tile(np.asarray(inputs["out_b"], np.float32), T).reshape(30, 1)
    # odd-chunk swapped gate layouts for the ts0 cc pair-packing
    encxw0o = np.concatenate([encxw0[:, 64:128], encxw0[:, 0:64]], axis=1)
    decw0o = np.concatenate([decw0[:, :, 64:128], decw0[:, :, 0:64]], axis=2)

    shared = {
        "encw": encw.reshape(128, 5 * 256).astype(BFP),
        "decw0": decw0.reshape(128, 5 * 256).astype(BFP),
        "encxw0": encxw0.astype(BFP),
        "encxw0o": encxw0o.astype(BFP),
        "decw0o": decw0o.reshape(128, 5 * 128).astype(BFP),
        "decw": decw.reshape(128, 9 * 256).astype(BFP),
        "w3": w3.reshape(128, 45 * 30).astype(BFP),
        "aps": aps,
        "bout": bout,
    }
    return [dict(shared, xim=np.ascontiguousarray(xim[c])) for c in range(8)]


_CACHE = {}


def kernel(**inputs):
    if "nc" not in _CACHE:
        _CACHE["nc"] = _build()
    nc = _CACHE["nc"]
    in_maps = _prep(inputs)
    from concourse.bass_utils import run_bass_kernel_spmd
    res = run_bass_kernel_spmd(nc, in_maps, core_ids=list(range(8)))
    kernel.last_exec_ns = res.exec_time_ns
    y = np.stack([
        np.asarray(res.results[c]["y"], np.float32)
        .reshape(T, 3, H, W).transpose(0, 2, 3, 1)
        for c in range(8)
    ])
    return y


# revision 11
# speedup vs baseline: 1.0176x; 1.0002x over previous
"""PredRNN (ConvLSTM enc -> BN -> ConvLSTM dec -> BN -> Conv3D -> sigmoid) on 8 trn2 cores.

Sharding: data-parallel over batch (B=8), one sample per core. Per core:
channel-partition layout, 3x3 convs as shift-window matmuls from padded
bf16 image buffers. Encoder h is stored twice per parity tile ([h; h up1]
in partition halves) so ky-taps pair into K=128 matmuls. Enc t>0 runs at
the 5-pass floor per PSUM half: 3 direct pairs (ky0+ky1 taps), one staged
pair (ky2 kx0+kx1 via column-shifted Pool copies), and one staged K=91
matmul merging the ky2 kx2 tap with the 27-row x-conv (x DMA'd straight
into the staging tile). Staging copies are emitted 2 chunks ahead (and
pre-staged across timestep boundaries) so the Pool queue stays in front
of the PE. Decoder uses K=128 fused [e|h_dec] taps (9/chunk, optimal).
PE pass count 9888 = the bf16 structural floor for this layout.
enc(b) and dec(b-2) are interleaved at every timestep: the enc phase
alone is Act-bound (2448 vs 2133 ns/chunk) while dec has PE slack, so
pairing keeps the whole timestep PE-bound; at ts9 the last chunks are
de-interleaved so the dec-only tail drains the eviction backlog before
conv3d WARs on all 8 PSUM banks. ts0 skips all zero-state h convolutions
(enc: x-conv only; dec: 5-pass e-only conv via decw0: 3 pairs + staged
pair + direct ky2 single) and uses a ts0-only gate packing [i|o],[cc|-]
(f unused) so both relu gates evict in one act op. Gates evict from PSUM
with fused hard-sigmoid, all temps bf16, cell state bf16 in SBUF.
DMA-issue placement matters (Pool SWDGE issues cost ~1us of engine time):
dstage writes + border zeroing + D01 prefetch ride SP/HWDGE, weight loads
avoid the Act queue, dstage border zeroing is spread over t=1..8, and the
D01 prefetch is split into row-bands so no 6us transfer monopolizes the
DMA engines. State-image border memsets are deferred into the ts0 loop
except A0 row 0 / B0 (read early or data-hazardous). Conv3D reuses the
dead state buffers via tag-reuse (A1/B0 prefetched during ts9, A0/B1/ct
after), runs q-outer over 8-block PSUM groups on the pA/pB bank
rotations; sigmoid eviction. Final-timestep state writes are skipped.
t_if (the sole pA reader, heading every eviction chain) rotates 3 deep
so PSUM bank recycling decouples from the DVE chain at phase drains.
ts0 is elementwise-bound (no h-convs), so it runs PAIRED: the cc gates of
chunks (2p, 2p+1) share one PSUM bank at opposite partition halves (odd
chunks use swapped [o|i] weight/bias layouts: encxw0o/decw0o/aps cols
12-13) so a single tanh act evicts both, and the ts0-only e-up image copy
is a Pool tensor_copy of the just-written e rows instead of a second DVE
affine - balancing Act/DVE/Pool just above the PE pace.
"""
import sys

sys.path.insert(0, "/opt/trn_rl_repo")
import numpy as np
import ml_dtypes

import concourse.bass as bass
import concourse.tile as tile
from concourse import mybir
from concourse.vector_clock import ScopedClock

BF16 = mybir.dt.bfloat16
F32 = mybir.dt.float32
AF = mybir.ActivationFunctionType
ALU = mybir.AluOpType

T, H, W, F, C = 10, 128, 128, 64, 3
PW = H + 2
NBLK = H // 4
NPIX = H * W
BN_EPS = 1e-3
BFP = ml_dtypes.bfloat16


def _patched_drain_and_barrier(self, tick_clock, wait_clock):
    nc = self.nc
    carrier = nc.sync.nop(nofuse=True, hint="drain_waits")
    wait_clock.add_sem_waits(carrier.ins, ScopedClock({None: tick_clock.global_clock}))
    si = carrier.ins.sync_info
    waits = list(si.on_wait) if si is not None else []
    if len(waits) > 1:
        si.on_wait = waits[:1]
        for w in waits[1:]:
            n = nc.sync.nop(nofuse=True, hint="drain_waits")
            n.ins.sync_info = mybir.SyncInfo(on_wait=[w], on_update=[])
    nc.sync.drain()
    nc.all_engine_barrier()
    popped = nc._tile_sem_poison_stack.pop()
    assert popped is self._sem_poison
    nc.clear_and_free_semaphores(list(self.sems.allocated().values()))
    nc.all_engine_barrier()


tile.TileContext._drain_and_barrier = _patched_drain_and_barrier


def split_multi_waits(nc, max_keep=1):
    """Walrus codegen rejects >1 sem wait on compute instructions; hoist
    extras onto same-engine single-wait NOPs inserted just before."""
    n_split = 0
    for fn in nc.m.functions:
        for blk in fn.blocks:
            insts = blk.instructions
            i = 0
            while i < len(insts):
                inst = insts[i]
                si = inst.sync_info
                waits = list(si.on_wait) if si is not None and si.on_wait else []
                if len(waits) > max_keep:
                    for j, w in enumerate(waits[:-max_keep]):
                        nop = mybir.InstNoOp(
                            name=f"{inst.name}_w{j}",
                            engine=inst.engine,
                            sync_info=mybir.SyncInfo(on_wait=[w], on_update=[]),
                            bass_nofuse=True,
                            ins=[],
                            outs=[],
                        )
                        insts.insert(i, nop)
                        i += 1
                    si.on_wait = waits[-max_keep:]
                    n_split += 1
                i += 1
    return n_split


def _build(do_split=True):
    nc = bass.Bass()
    xim_d = nc.dram_tensor("xim", [T, 27, NPIX], BF16, kind="ExternalInput")
    encw_d = nc.dram_tensor("encw", [128, 5 * 256], BF16, kind="ExternalInput")
    decw0_d = nc.dram_tensor("decw0", [128, 5 * 256], BF16, kind="ExternalInput")
    encxw0_d = nc.dram_tensor("encxw0", [27, 256], BF16, kind="ExternalInput")
    encxw0o_d = nc.dram_tensor("encxw0o", [27, 128], BF16, kind="ExternalInput")
    decw0o_d = nc.dram_tensor("decw0o", [128, 5 * 128], BF16, kind="ExternalInput")
    decw_d = nc.dram_tensor("decw", [128, 9 * 256], BF16, kind="ExternalInput")
    w3_d = nc.dram_tensor("w3", [128, 45 * 30], BF16, kind="ExternalInput")
    aps_d = nc.dram_tensor("aps", [128, 14], F32, kind="ExternalInput")
    bout_d = nc.dram_tensor("bout", [30, 1], F32, kind="ExternalInput")
    dstage = nc.dram_tensor("dstage", [T, F, PW, PW], BF16, kind="Internal")
    y_d = nc.dram_tensor("y", [30, H, W], F32, kind="ExternalOutput")

    with tile.TileContext(nc) as tc:
        with tc.tile_pool(name="wp", bufs=1) as wp:
            encw = wp.tile([128, 5, 256], BF16)
            encxw0 = wp.tile([27, 256], BF16)
            encxw0o = wp.tile([27, 128], BF16)
            decw0o = wp.tile([128, 5, 128], BF16)
            decw = wp.tile([128, 9, 256], BF16)
            decw0 = wp.tile([128, 5, 256], BF16)
            w3t = wp.tile([128, 45, 30], BF16)
            aps = wp.tile([128, 14], F32)
            bout = wp.tile([30, 1], F32)
            zb = wp.tile([64, PW], BF16)
            # ordered by first use: ts0 needs encxw0+aps immediately, decw0 at
            # dec-ts0, encw/decw only from ts1, w3t/bout at conv3d
            nc.sync.dma_start(encxw0[:], encxw0_d[:])
            nc.scalar.dma_start(encxw0o[:], encxw0o_d[:])
            # aps rides the Act queue (it is Act's own first need, ~3us in);
            # decw0 is issued after the first xs loads inside the ts0 loop
            # path (SP queue order: encxw0, xs(0), decw0 via emission below)
            nc.scalar.dma_start(aps[:], aps_d[:])
            nc.gpsimd.dma_start(encw[:], encw_d[:].rearrange("p (s g) -> p s g", s=5))
            nc.gpsimd.dma_start(decw[:], decw_d[:].rearrange("p (s g) -> p s g", s=9))
            nc.gpsimd.dma_start(w3t[:], w3_d[:].rearrange("p (s g) -> p s g", s=45))
            nc.gpsimd.dma_start(bout[:], bout_d[:])
            nc.vector.memset(zb[:], 0.0)

            with tc.tile_pool(name="sp", bufs=1) as sp, \
                 tc.tile_pool(name="tp", bufs=1) as tp, \
                 tc.tile_pool(name="pp", bufs=1, space=bass.MemorySpace.PSUM) as pp:
                # state tiles carry tags so conv3d can reuse their buffers
                # (tag-reuse = same space, deps tracked) once they go dead
                A0 = sp.tile([128, PW, PW], BF16, tag="A0", name="A0")
                A1 = sp.tile([128, PW, PW], BF16, tag="A1", name="A1")
                B0 = sp.tile([128, PW, PW], BF16, tag="B0", name="B0")
                B1 = sp.tile([128, PW, PW], BF16, tag="B1", name="B1")
                # cell state allocated at PW*PW (not NPIX) so its buffer can
                # host a conv3d frame pair later; only [:, :NPIX] is used
                ct = sp.tile([128, PW * PW], BF16, tag="ct", name="ct")
                # ts0 skips all h-reads, so only the never-written borders of
                # the state images need zeroing (pad rows/cols + bottom row
                # 128). B0 first: dec-ts0 reads it within ~10us; the other
                # tiles aren't read before t=1, so their memsets are emitted
                # lazily inside the ts0 loop (see below) to keep the DVE
                # queue clear for the first evictions.
                # on Pool (idle at startup; DVE would delay the first
                # eviction chains). Pool memset runs at full efficiency.
                for im in (B0,):
                    nc.gpsimd.memset(im[:, 0, :], 0.0)
                    nc.gpsimd.memset(im[:, 128:130, :], 0.0)
                    nc.gpsimd.memset(im[:, :, 0], 0.0)
                    nc.gpsimd.memset(im[:, :, PW - 1], 0.0)
                # A0's bottom-half row 0 is DATA (enc(0,b=0)'s rowup copy
                # writes rows 0:4): this memset must precede that write, so
                # it cannot be deferred with the other lazy strips
                nc.gpsimd.memset(A0[:, 0, :], 0.0)
                # ct needs no memset: every element is written at ts0 (c0 = i*cc)
                # before any read. dstage border zeroing is spread over
                # t=1..8 on SP (only needs to precede the conv3d reads).
                Bs = [B0, B1]
                As = [A0, A1]

                def tmp(tag):
                    # t_if heads every eviction chain (sole pA reader): give
                    # it a deeper rotation so its WAR trails 3 chunk-pairs
                    # instead of 2, decoupling PSUM recycling at drains
                    return tp.tile([128, 512], BF16, tag=tag,
                                   bufs=(3 if tag == "t_if" else 2), name=tag)

                def enc_stage(b, t, Aprev):
                    # ky2 taps relocated/column-shifted into K=128-packable
                    # tiles; x rides partitions 64:91 of s2 (K=91 pass).
                    # Aprev bottom half is h stored rowup: window rows
                    # r0+1:r0+5 = tap ky2, col offset = kx.
                    r0, c0 = 4 * b, 512 * b
                    s1 = tp.tile([128, 4, 128], BF16, tag="s1", bufs=4)
                    s2 = tp.tile([128, 4, 128], BF16, tag="s2", bufs=4)
                    nc.gpsimd.tensor_copy(s1[0:64, :, :], Aprev[64:128, r0 + 1:r0 + 5, 0:128])
                    nc.gpsimd.tensor_copy(s1[64:128, :, :], Aprev[64:128, r0 + 1:r0 + 5, 1:129])
                    nc.gpsimd.tensor_copy(s2[0:64, :, :], Aprev[64:128, r0 + 1:r0 + 5, 2:130])
                    nc.sync.dma_start(
                        s2[64:91, :, :],
                        xim_d[t, :, 512 * b:512 * b + 512].rearrange("p (a b) -> p a b", a=4))
                    return s1, s2

                D01 = []
                for t in range(T):
                    Acur, Aprev = As[t % 2], As[(t - 1) % 2]
                    Bcur, Bnext = Bs[t % 2], Bs[(t + 1) % 2]

                    # ---------------- encoder ----------------
                    def enc_block(b, stage=None, t=t, Acur=Acur, Aprev=Aprev, Bcur=Bcur):
                        r0, c0 = 4 * b, 512 * b
                        if t == 0:
                            xs = tp.tile([27, 512], BF16, tag="s1", bufs=4)
                            nc.sync.dma_start(xs[:], xim_d[t, :, c0:c0 + 512])
                        else:
                            s1, s2 = stage
                        pA = pp.tile([128, 512], F32, tag="pA", bufs=4)
                        pB = pp.tile([128, 512], F32, tag="pB", bufs=4)
                        for ch, ps in ((0, pA), (1, pB)):
                            if t > 0:  # h_{-1}=0: ts0 needs only the x conv
                                for kx in range(3):
                                    # K=128 pair: ky=0 (top) + ky=1 (bottom)
                                    nc.tensor.matmul(
                                        ps[:],
                                        encw[:, kx, 128 * ch:128 * ch + 128],
                                        Aprev[:, r0:r0 + 4, kx:kx + 128],
                                        start=(kx == 0), stop=False)
                                # K=128 staged pair: taps (2,0)+(2,1)
                                nc.tensor.matmul(
                                    ps[:], encw[:, 3, 128 * ch:128 * ch + 128],
                                    s1[:, :, :], start=False, stop=False)
                                # K=91: tap (2,2) + 27-row x-conv
                                nc.tensor.matmul(
                                    ps[:], encw[0:91, 4, 128 * ch:128 * ch + 128],
                                    s2[0:91, :, :], start=False, stop=True)
                            else:
                                nc.tensor.matmul(
                                    ps[:], encxw0[:, 128 * ch:128 * ch + 128], xs[:],
                                    start=True, stop=True)
                        t_if, t_tc, t_o = tmp("t_if"), tmp("t_tc"), tmp("t_o")
                        t_s, t_s2, t_th, t_h = tmp("t_s"), tmp("t_s2"), tmp("t_th"), tmp("t_h")
                        cblk = ct[64:128, c0:c0 + 512]
                        if t == 0:
                            # ts0: f unused (c_{-1}=0); gates packed [i|o],[cc|-]
                            # so both relu gates evict in ONE act op
                            nc.scalar.activation(t_if[:], pA[:], AF.Relu, bias=aps[:, 8:9], scale=0.2)
                            nc.scalar.activation(t_tc[0:64, :], pB[0:64, :], AF.Tanh, bias=aps[0:64, 9:10], scale=1.0)
                            nc.vector.scalar_tensor_tensor(cblk, t_if[0:64, :], 1.0, t_tc[0:64, :], ALU.min, ALU.mult)
                            # t_th on partitions 64:128 so the STT inputs
                            # (o-gate at 64:128) share a base partition
                            nc.scalar.activation(t_th[64:128, :], cblk, AF.Tanh)
                            nc.vector.scalar_tensor_tensor(t_h[0:64, :], t_if[64:128, :], 1.0, t_th[64:128, :], ALU.min, ALU.mult)
                        else:
                            nc.scalar.activation(t_if[:], pA[:], AF.Relu, bias=aps[:, 0:1], scale=0.2)
                            nc.scalar.activation(t_tc[0:64, :], pB[0:64, :], AF.Tanh, bias=aps[0:64, 2:3], scale=1.0)
                            nc.scalar.activation(t_o[0:64, :], pB[64:128, :], AF.Relu, bias=aps[64:128, 2:3], scale=0.2)
                            nc.vector.scalar_tensor_tensor(t_s[64:128, :], t_if[0:64, :], 1.0, t_tc[0:64, :], ALU.min, ALU.mult)
                            nc.vector.scalar_tensor_tensor(t_s2[64:128, :], t_if[64:128, :], 1.0, cblk, ALU.min, ALU.mult)
                            nc.vector.tensor_tensor(cblk, t_s[64:128, :], t_s2[64:128, :], ALU.add)
                            nc.scalar.activation(t_th[0:64, :], cblk, AF.Tanh)
                            nc.vector.scalar_tensor_tensor(t_h[0:64, :], t_o[0:64, :], 1.0, t_th[0:64, :], ALU.min, ALU.mult)
                        hr = t_h[0:64, :].rearrange("p (a b) -> p a b", a=4)
                        if t < T - 1:  # ts9's h_enc is never convolved again
                            nc.gpsimd.tensor_copy(Acur[0:64, r0 + 1:r0 + 5, 1:1 + W], hr)
                            nc.gpsimd.tensor_copy(Acur[64:128, r0:r0 + 4, 1:1 + W], hr)
                        nc.vector.tensor_scalar(
                            Bcur[0:64, r0 + 1:r0 + 5, 1:1 + W], hr,
                            aps[0:64, 4:5], aps[0:64, 5:6], ALU.mult, ALU.add)
                        if t == 0:  # also e down1 into h-half for paired dec ts0
                            nc.vector.tensor_scalar(
                                Bcur[64:128, r0:r0 + 4, 1:1 + W], hr,
                                aps[0:64, 4:5], aps[0:64, 5:6], ALU.mult, ALU.add)

                    def dec_stage0(b, Bcur=Bcur):
                        # ts0 e-only conv: stage the (2,0)+(2,1) pair from the
                        # e-up half of B (column shifts baked into the copies)
                        r0 = 4 * b
                        s1 = tp.tile([128, 4, 128], BF16, tag="s2", bufs=4)
                        nc.gpsimd.tensor_copy(s1[0:64, :, :], Bcur[64:128, r0 + 1:r0 + 5, 0:128])
                        nc.gpsimd.tensor_copy(s1[64:128, :, :], Bcur[64:128, r0 + 1:r0 + 5, 1:129])
                        return s1

                    # ---------------- decoder ----------------
                    def dec_block(b, stage=None, t=t, Bcur=Bcur, Bnext=Bnext):
                        r0, c0 = 4 * b, 512 * b
                        pA = pp.tile([128, 512], F32, tag="pA", bufs=4)
                        pB = pp.tile([128, 512], F32, tag="pB", bufs=4)
                        for ch, ps in ((0, pA), (1, pB)):
                            if t == 0:
                                # h_dec_{-1}=0; Bcur holds [e; e up1] -> pair kys
                                for kx in range(3):
                                    nc.tensor.matmul(
                                        ps[:],
                                        decw0[:, kx, 128 * ch:128 * ch + 128],
                                        Bcur[:, r0:r0 + 4, kx:kx + 128],
                                        start=(kx == 0), stop=False)
                                # staged pair (2,0)+(2,1)
                                nc.tensor.matmul(
                                    ps[:], decw0[:, 3, 128 * ch:128 * ch + 128],
                                    stage[:, :, :], start=False, stop=False)
                                # direct K=64 single (2,2)
                                nc.tensor.matmul(
                                    ps[:], decw0[64:128, 4, 128 * ch:128 * ch + 128],
                                    Bcur[64:128, r0 + 1:r0 + 5, 2:130],
                                    start=False, stop=True)
                            else:
                                for s in range(9):
                                    ky, kx = s // 3, s % 3
                                    nc.tensor.matmul(
                                        ps[:],
                                        decw[:, s, 128 * ch:128 * ch + 128],
                                        Bcur[:, r0 + ky:r0 + ky + 4, kx:kx + 128],
                                        start=(s == 0), stop=(s == 8))
                        t_if, t_tc, t_o = tmp("t_if"), tmp("t_tc"), tmp("t_o")
                        t_s, t_s2, t_th, t_h = tmp("t_s"), tmp("t_s2"), tmp("t_th"), tmp("t_h")
                        cblk = ct[0:64, c0:c0 + 512]
                        if t == 0:
                            # ts0: f unused; decw0 gates packed [i|o],[cc|-]
                            # so both relu gates evict in ONE act op
                            nc.scalar.activation(t_if[:], pA[:], AF.Relu, bias=aps[:, 10:11], scale=0.2)
                            nc.scalar.activation(t_tc[0:64, :], pB[0:64, :], AF.Tanh, bias=aps[0:64, 11:12], scale=1.0)
                            nc.vector.scalar_tensor_tensor(cblk, t_if[0:64, :], 1.0, t_tc[0:64, :], ALU.min, ALU.mult)
                            nc.scalar.activation(t_th[64:128, :], cblk, AF.Tanh)
                            nc.vector.scalar_tensor_tensor(t_h[0:64, :], t_if[64:128, :], 1.0, t_th[64:128, :], ALU.min, ALU.mult)
                        else:
                            # chunk A is [f|i] (host-permuted columns)
                            nc.scalar.activation(t_if[:], pA[:], AF.Relu, bias=aps[:, 1:2], scale=0.2)
                            nc.scalar.activation(t_tc[64:128, :], pB[0:64, :], AF.Tanh, bias=aps[0:64, 3:4], scale=1.0)
                            nc.scalar.activation(t_o[0:64, :], pB[64:128, :], AF.Relu, bias=aps[64:128, 3:4], scale=0.2)
                            nc.vector.scalar_tensor_tensor(t_s2[0:64, :], t_if[0:64, :], 1.0, cblk, ALU.min, ALU.mult)
                            nc.vector.scalar_tensor_tensor(t_s[0:64, :], t_if[64:128, :], 1.0, t_tc[64:128, :], ALU.min, ALU.mult)
                            nc.vector.tensor_tensor(cblk, t_s[0:64, :], t_s2[0:64, :], ALU.add)
                            nc.scalar.activation(t_th[0:64, :], cblk, AF.Tanh)
                            nc.vector.scalar_tensor_tensor(t_h[0:64, :], t_o[0:64, :], 1.0, t_th[0:64, :], ALU.min, ALU.mult)
                        hr = t_h[0:64, :].rearrange("p (a b) -> p a b", a=4)
                        if t < T - 1:  # ts9's h_dec feeds no further timestep
                            nc.gpsimd.tensor_copy(Bnext[64:128, r0 + 1:r0 + 5, 1:1 + W], hr)
                        dtmp = tp.tile([64, 512], BF16, tag="dtmp", bufs=2)
                        nc.vector.tensor_scalar(
                            dtmp[:], t_h[0:64, :],
                            aps[0:64, 6:7], aps[0:64, 7:8], ALU.mult, ALU.add)
                        # SP-issued (HWDGE ~650ns) instead of Pool (SWDGE ~1us):
                        # keeps the Pool queue free for staging + h-writes
                        nc.sync.dma_start(
                            dstage[t, :, r0 + 1:r0 + 5, 1:1 + W],
                            dtmp[:].rearrange("p (a b) -> p a b", a=4))

                    # ---------------- ts0 pair blocks ----------------
                    # cc gates of a chunk pair share ONE PSUM bank at opposite
                    # partition halves (odd chunks use swapped [o|i] weights +
                    # bias columns), so a single 612ns tanh evicts both ccs.
                    # Even chunks' c-STT runs on Pool to balance DVE.
                    def load_xs0(p):
                        # prefetched one pair ahead: the SP queue serializes
                        # ~650ns/issue, so just-in-time loads starve the PE
                        # during the first pairs
                        tiles = []
                        for j in (0, 1):
                            c0 = 512 * (2 * p + j)
                            xs = tp.tile([27, 512], BF16, tag="s1", bufs=4)
                            nc.sync.dma_start(xs[:], xim_d[0, :, c0:c0 + 512])
                            tiles.append(xs)
                        return tiles

                    def enc_pair0(p, xsp, Acur=Acur, Bcur=Bcur):
                        pB = pp.tile([128, 512], F32, tag="pB", bufs=4)
                        gates = []
                        for j in (0, 1):
                            b = 2 * p + j
                            c0 = 512 * b
                            xs = xsp[j]
                            pA = pp.tile([128, 512], F32, tag="pA", bufs=4)
                            nc.tensor.matmul(
                                pA[:],
                                encxw0[:, 0:128] if j == 0 else encxw0o[:],
                                xs[:], start=True, stop=True)
                            nc.tensor.matmul(
                                pB[64 * j:64 * j + 64, :], encxw0[:, 128:192],
                                xs[:], start=True, stop=True)
                            t_if = tmp("t_if")
                            nc.scalar.activation(
                                t_if[:], pA[:], AF.Relu,
                                bias=aps[:, 8 + 4 * j:9 + 4 * j], scale=0.2)
                            gates.append(t_if)
                        t_tc = tmp("t_tc")
                        nc.scalar.activation(t_tc[:], pB[:], AF.Tanh,
                                             bias=aps[:, 9:10], scale=1.0)
                        for j in (0, 1):
                            b = 2 * p + j
                            r0, c0 = 4 * b, 512 * b
                            t_if = gates[j]
                            cblk = ct[64:128, c0:c0 + 512]
                            i_sl = slice(64 * j, 64 * j + 64)
                            o_sl = slice(64 - 64 * j, 128 - 64 * j)
                            nc.vector.scalar_tensor_tensor(
                                cblk, t_if[i_sl, :], 1.0, t_tc[i_sl, :],
                                ALU.min, ALU.mult)
                            t_th, t_h = tmp("t_th"), tmp("t_h")
                            nc.scalar.activation(t_th[o_sl, :], cblk, AF.Tanh)
                            nc.vector.scalar_tensor_tensor(
                                t_h[0:64, :], t_if[o_sl, :], 1.0, t_th[o_sl, :],
                                ALU.min, ALU.mult)
                            hr = t_h[0:64, :].rearrange("p (a b) -> p a b", a=4)
                            nc.gpsimd.tensor_copy(Acur[0:64, r0 + 1:r0 + 5, 1:1 + W], hr)
                            nc.gpsimd.tensor_copy(Acur[64:128, r0:r0 + 4, 1:1 + W], hr)
                            nc.vector.tensor_scalar(
                                Bcur[0:64, r0 + 1:r0 + 5, 1:1 + W], hr,
                                aps[0:64, 4:5], aps[0:64, 5:6], ALU.mult, ALU.add)
                            # e-up = copy of the e rows just written (Pool,
                            # not a second DVE affine: DVE is the ts0 binder)
                            nc.gpsimd.tensor_copy(
                                Bcur[64:128, r0:r0 + 4, 1:1 + W],
                                Bcur[0:64, r0 + 1:r0 + 5, 1:1 + W])

                    def dec_pair0(p, st2, Bcur=Bcur, Bnext=Bnext):
                        pB = pp.tile([128, 512], F32, tag="pB", bufs=4)
                        gates = []
                        for j in (0, 1):
                            b = 2 * p + j
                            r0 = 4 * b
                            st = st2[j]
                            pA = pp.tile([128, 512], F32, tag="pA", bufs=4)
                            wA = decw0 if j == 0 else decw0o
                            for kx in range(3):
                                nc.tensor.matmul(
                                    pA[:], wA[:, kx, 0:128],
                                    Bcur[:, r0:r0 + 4, kx:kx + 128],
                                    start=(kx == 0), stop=False)
                                nc.tensor.matmul(
                                    pB[64 * j:64 * j + 64, :], decw0[:, kx, 128:192],
                                    Bcur[:, r0:r0 + 4, kx:kx + 128],
                                    start=(kx == 0), stop=False)
                            nc.tensor.matmul(
                                pA[:], wA[:, 3, 0:128], st[:, :, :],
                                start=False, stop=False)
                            nc.tensor.matmul(
                                pB[64 * j:64 * j + 64, :], decw0[:, 3, 128:192],
                                st[:, :, :], start=False, stop=False)
                            nc.tensor.matmul(
                                pA[:], wA[64:128, 4, 0:128],
                                Bcur[64:128, r0 + 1:r0 + 5, 2:130],
                                start=False, stop=True)
                            nc.tensor.matmul(
                                pB[64 * j:64 * j + 64, :], decw0[64:128, 4, 128:192],
                                Bcur[64:128, r0 + 1:r0 + 5, 2:130],
                                start=False, stop=True)
                            t_if = tmp("t_if")
                            nc.scalar.activation(
                                t_if[:], pA[:], AF.Relu,
                                bias=aps[:, 10 + 3 * j:11 + 3 * j], scale=0.2)
                            gates.append(t_if)
                        t_tc = tmp("t_tc")
                        nc.scalar.activation(t_tc[:], pB[:], AF.Tanh,
                                             bias=aps[:, 11:12], scale=1.0)
                        for j in (0, 1):
                            b = 2 * p + j
                            r0, c0 = 4 * b, 512 * b
                            t_if = gates[j]
                            cblk = ct[0:64, c0:c0 + 512]
                            i_sl = slice(64 * j, 64 * j + 64)
                            o_sl = slice(64 - 64 * j, 128 - 64 * j)
                            nc.vector.scalar_tensor_tensor(
                                cblk, t_if[i_sl, :], 1.0, t_tc[i_sl, :],
                                ALU.min, ALU.mult)
                            t_th, t_h = tmp("t_th"), tmp("t_h")
                            nc.scalar.activation(t_th[o_sl, :], cblk, AF.Tanh)
                            nc.vector.scalar_tensor_tensor(
                                t_h[0:64, :], t_if[o_sl, :], 1.0, t_th[o_sl, :],
                                ALU.min, ALU.mult)
                            hr = t_h[0:64, :].rearrange("p (a b) -> p a b", a=4)
                            nc.gpsimd.tensor_copy(Bnext[64:128, r0 + 1:r0 + 5, 1:1 + W], hr)
                            dtmp = tp.tile([64, 512], BF16, tag="dtmp", bufs=2)
                            nc.vector.tensor_scalar(
                                dtmp[:], t_h[0:64, :],
                                aps[0:64, 6:7], aps[0:64, 7:8], ALU.mult, ALU.add)
                            nc.sync.dma_start(
                                dstage[0, :, r0 + 1:r0 + 5, 1:1 + W],
                                dtmp[:].rearrange("p (a b) -> p a b", a=4))

                    if t == 0:
                        # coarse interleave: dec blocks (PE-heavy) fill the
                        # PE while enc evictions (act-paced) run; offset 4
                        # keeps dec eviction chains from head-blocking the
                        # act queue (dec(k) needs enc(k+1)'s bottom row)
                        dstages = {}
                        # A0/A1/B1 borders aren't read before t=1: emit their
                        # memsets lazily mid-ts0 to keep the DVE queue clear
                        # for the first eviction chains
                        lazy_ms = []
                        for im in (A0, A1, B1):
                            lazy_ms += [
                                lambda im=im: nc.vector.memset(im[:, 128:130, :], 0.0),
                                lambda im=im: nc.vector.memset(im[:, :, 0], 0.0),
                                lambda im=im: nc.vector.memset(im[:, :, PW - 1], 0.0),
                            ]
                            if im is not A0:  # A0 row 0 is set upfront (data hazard)
                                lazy_ms.append(
                                    lambda im=im: nc.vector.memset(im[:, 0, :], 0.0))
                        NP = NBLK // 2
                        xs_pend = {0: load_xs0(0)}
                        for p in range(NP):
                            if p + 1 < NP:
                                xs_pend[p + 1] = load_xs0(p + 1)
                            enc_pair0(p, xs_pend.pop(p))
                            if p == 1:
                                # behind the first three xs pairs in the SP
                                # queue; dec pair 0 needs these only at ~12us
                                nc.sync.dma_start(
                                    decw0[:],
                                    decw0_d[:].rearrange("p (s g) -> p s g", s=5))
                            if p == 2:
                                nc.sync.dma_start(
                                    decw0o[:],
                                    decw0o_d[:].rearrange("p (s g) -> p s g", s=5))
                            if 3 <= p < 3 + len(lazy_ms):
                                lazy_ms[p - 3]()
                            if p >= 1:
                                dstages[2 * p - 1] = dec_stage0(2 * p - 1)
                            if p + 1 < NP:
                                # chunk 2p needs only enc chunk 2p+1 (this
                                # pair): emit before next pair's Pool writes
                                dstages[2 * p] = dec_stage0(2 * p)
                            if p >= 2:
                                k = 2 * (p - 2)
                                dec_pair0(p - 2, [dstages.pop(k), dstages.pop(k + 1)])
                        # tail: stage emissions interleaved with consumers so
                        # slot reuse never head-blocks the Pool queue
                        dstages[NBLK - 2] = dec_stage0(NBLK - 2)
                        dec_pair0(NP - 2, [dstages.pop(NBLK - 4), dstages.pop(NBLK - 3)])
                        dstages[NBLK - 1] = dec_stage0(NBLK - 1)
                        dec_pair0(NP - 1, [dstages.pop(NBLK - 2), dstages.pop(NBLK - 1)])
                        # cross-boundary pipelining: run enc(1, 0/1) now
                        # (their deps - A0, stages - are ready) so their PSUM
                        # groups allocate before the ts0 eviction backlog
                        # drains; t=1's loop then starts at chunk 2
                        for j in range(4):
                            sj = enc_stage(j, 1, Acur)
                            enc_block(j, sj, t=1, Acur=As[1], Aprev=As[0],
                                      Bcur=Bs[1])
                        pend = {4: enc_stage(4, 1, Acur), 5: enc_stage(5, 1, Acur)}
                        continue
                    # dstage borders for earlier frames (conv3d needs them;
                    # nothing reads them before ts9): ~1 frame per timestep
                    bframes = [t - 1] if t < 8 else ([7, 8, 9] if t == 8 else [])
                    for bf in bframes:
                        nc.sync.dma_start(dstage[bf, :, 0, :], zb[:])
                        nc.sync.dma_start(dstage[bf, :, PW - 1, :], zb[:])
                        nc.sync.dma_start(dstage[bf, :, :, 0], zb[:])
                        nc.sync.dma_start(dstage[bf, :, :, PW - 1], zb[:])
                    stages = pend
                    if t == 1:
                        # reclaim B0 bottom row 0 (junked by ts0's e-down1);
                        # dec(1) writes B0 rows 1..128 only
                        nc.vector.memset(Bnext[64:128, 0, :], 0.0)
                    if t == T - 1:
                        # A1/B0 are dead through ts9 (final-ts writes
                        # skipped, last reads at ts8): prefetch conv3d
                        # frames 0..3 into their buffers during dec ts9.
                        # Split into row-bands so no single 6us transfer
                        # monopolizes the DMA engines against the small
                        # latency-critical stage/dstage transfers.
                        D01 = [
                            sp.tile([128, PW, PW], BF16, tag="A1", name="Dp0"),
                            sp.tile([128, PW, PW], BF16, tag="B0", name="Dp1"),
                        ]
                        d01_parts = []
                        nband = 4
                        rb = [0, 33, 66, 99, PW]
                        for q in range(2):
                            for hh, h0 in ((0, 0), (64, 1)):
                                for k in range(nband):
                                    d01_parts.append((q, hh, rb[k], rb[k + 1], h0))
                    # interleave enc(b) with dec(b-2): the enc phase alone is
                    # act-bound (2448 > 2133 ns/chunk) while dec has PE slack;
                    # pairing keeps every chunk-pair PE-bound. dec(k) needs
                    # e rows through 4k+4, written by enc(k+1).
                    def dec_k(k, t=t):
                        if t == T - 1 and k < len(d01_parts):
                            q, hh, ra, rz, h0 = d01_parts[k]
                            nc.sync.dma_start(
                                D01[q][hh:hh + 64, ra:rz, :],
                                dstage[2 * q + h0, :, ra:rz, :])
                        dec_block(k)
                    # at t==T-1, de-interleave the last chunks: the dec-only
                    # tail has PE slack (3840 vs 2448 ns/chunk), letting the
                    # Act/DVE eviction backlog drain before conv3d WARs on
                    # all 8 PSUM banks (else ~10us PE stall at conv3d start)
                    ilv_last = NBLK - 2 if t < T - 1 else 28
                    # leading chunks of this timestep already ran during
                    # the previous timestep's tail (cross-boundary pipelining;
                    # 4 chunks across the deep ts0 drain, 2 elsewhere)
                    bstart = 4 if t == 1 else 2
                    if t == 1:
                        dec_k(0)
                        dec_k(1)
                    for b in range(bstart, NBLK):
                        if b + 2 < NBLK:
                            stages[b + 2] = enc_stage(b + 2, t, Aprev)
                        enc_block(b, stages.pop(b))
                        if 0 <= b - 2 < ilv_last:
                            dec_k(b - 2)
                    for k in range(ilv_last, NBLK):
                        dec_k(k)
                        if t < T - 1 and k >= NBLK - 2:
                            j = k - (NBLK - 2)
                            sj = enc_stage(j, t + 1, Acur)
                            enc_block(j, sj, t=t + 1, Acur=As[(t + 1) % 2],
                                      Aprev=As[t % 2], Bcur=Bs[(t + 1) % 2])
                            if k == NBLK - 1:
                                pend = {2: enc_stage(2, t + 1, Acur),
                                        3: enc_stage(3, t + 1, Acur)}

                # ---------------- conv3d + sigmoid ----------------
                # frames 0..3 prefetched during ts9 (D01); frames 4..9 load
                # into the now-dead A0/B1/ct buffers via tag reuse
                D = D01 + [
                    sp.tile([128, PW, PW], BF16, tag="A0", name="D2"),
                    sp.tile([128, PW, PW], BF16, tag="B1", name="D3"),
                    sp.tile([128, PW, PW], BF16, tag="ct", name="D4"),
                ]
                dma_engs = [nc.sync, nc.scalar, nc.gpsimd]
                for q in range(2, 5):
                    dma_engs[(2 * q) % 3].dma_start(
                        D[q][0:64, :, :], dstage[2 * q, :, :, :])
                    dma_engs[(2 * q + 1) % 3].dma_start(
                        D[q][64:128, :, :], dstage[2 * q + 1, :, :, :])
                # q-outer over 8-block groups: early matmuls need only D0
                # while later D tiles are still in flight; PSUM groups reuse
                # the pA/pB bank rotations (rows 0:30 of each bank)
                for g in range(NBLK // 8):
                    pys = [pp.tile([128, 512], F32,
                                   tag=("pA" if bb % 2 == 0 else "pB"), bufs=4,
                                   name=f"py{bb}") for bb in range(8)]
                    for q in range(5):
                        for bb in range(8):
                            r0 = 4 * (8 * g + bb)
                            for s in range(9):
                                ky, kx = s // 3, s % 3
                                nc.tensor.matmul(
                                    pys[bb][0:30, :], w3t[:, q * 9 + s, :],
                                    D[q][:, r0 + ky:r0 + ky + 4, kx:kx + 128],
                                    start=(q == 0 and s == 0),
                                    stop=(q == 4 and s == 8))
                    for bb in range(8):
                        r0 = 4 * (8 * g + bb)
                        ty = tp.tile([30, 512], F32, tag="ty", bufs=1)
                        nc.scalar.activation(ty[:], pys[bb][0:30, :], AF.Sigmoid,
                                             bias=bout[:], scale=1.0)
                        nc.scalar.dma_start(
                            y_d[:, r0:r0 + 4, :],
                            ty[:].rearrange("p (a b) -> p a b", a=4))

    if do_split:
        split_multi_waits(nc)
    nc.finalize()
    return nc


def _prep(inputs):
    x = np.asarray(inputs["x"], np.float32)
    xpad = np.zeros((8, T, PW, PW, C), np.float32)
    xpad[:, :, 1:1 + H, 1:1 + W, :] = x
    xim = np.empty((8, T, 27, NPIX), BFP)
    for ky in range(3):
        for kx in range(3):
            s = ky * 3 + kx
            v = xpad[:, :, ky:ky + H, kx:kx + W, :]
            xim[:, :, s * 3:s * 3 + 3, :] = (
                v.transpose(0, 1, 4, 2, 3).reshape(8, T, 3, NPIX).astype(BFP))

    enc_Wh = np.asarray(inputs["enc_Wh"], np.float32)
    enc_Wx = np.asarray(inputs["enc_Wx"], np.float32)
    dec_Wx = np.asarray(inputs["dec_Wx"], np.float32)
    dec_Wh = np.asarray(inputs["dec_Wh"], np.float32)
    out_W = np.asarray(inputs["out_W"], np.float32)

    encw = np.zeros((128, 5, 256), np.float32)
    decw = np.zeros((128, 9, 256), np.float32)
    perm = np.concatenate([np.arange(64, 128), np.arange(0, 64), np.arange(128, 256)])
    perm0 = np.concatenate([np.arange(0, 64), np.arange(192, 256),
                            np.arange(128, 192), np.arange(64, 128)])
    for s in range(9):
        ky, kx = s // 3, s % 3
        decw[0:64, s, :] = dec_Wx[ky, kx][:, perm]
        decw[64:128, s, :] = dec_Wh[ky, kx][:, perm]
    decw0 = np.zeros((128, 5, 256), np.float32)
    for kx in range(3):
        # paired matmul: top half = tap ky=0, bottom = tap ky=1 (e up1 copy)
        encw[0:64, kx, :] = enc_Wh[0, kx]
        encw[64:128, kx, :] = enc_Wh[1, kx]
        # ts0 decoder: e-only paired conv, gates packed [i|o|c|junk]
        decw0[0:64, kx, :] = dec_Wx[0, kx][:, perm0]
        decw0[64:128, kx, :] = dec_Wx[1, kx][:, perm0]
    # staged pair (2,0)+(2,1) and merged (2,2)+x slots
    encw[0:64, 3, :] = enc_Wh[2, 0]
    encw[64:128, 3, :] = enc_Wh[2, 1]
    encw[0:64, 4, :] = enc_Wh[2, 2]
    encw[64:91, 4, :] = enc_Wx.reshape(27, 256)
    decw0[0:64, 3, :] = dec_Wx[2, 0][:, perm0]
    decw0[64:128, 3, :] = dec_Wx[2, 1][:, perm0]
    decw0[64:128, 4, :] = dec_Wx[2, 2][:, perm0]
    encxw = enc_Wx.reshape(27, 256)

    w3 = np.zeros((45, 128, 30), np.float32)
    for q in range(5):
        for j in range(2):
            f = 2 * q + j
            for t in range(max(0, f - 1), min(T - 1, f + 1) + 1):
                dt = f - t + 1
                for s in range(9):
                    ky, kx = s // 3, s % 3
                    w3[q * 9 + s, 64 * j:64 * j + 64, 3 * t:3 * t + 3] = out_W[dt, ky, kx]
    w3 = w3.transpose(1, 0, 2)  # [128, 45, 30]

    enc_b = np.asarray(inputs["enc_b"], np.float32)
    dec_b = np.asarray(inputs["dec_b"], np.float32)
    s_e = np.asarray(inputs["enc_gamma"], np.float32) / np.sqrt(
        np.asarray(inputs["enc_var"], np.float32) + BN_EPS)
    t_e = np.asarray(inputs["enc_beta"], np.float32) - np.asarray(inputs["enc_mean"], np.float32) * s_e
    s_d = np.asarray(inputs["dec_gamma"], np.float32) / np.sqrt(
        np.asarray(inputs["dec_var"], np.float32) + BN_EPS)
    t_d = np.asarray(inputs["dec_beta"], np.float32) - np.asarray(inputs["dec_mean"], np.float32) * s_d

    # ts0 encoder x-conv with gate columns [i|o|c|0] (f unused at ts0)
    encxw0 = np.zeros((27, 256), np.float32)
    encxw0[:, 0:64] = encxw[:, 0:64]
    encxw0[:, 64:128] = encxw[:, 192:256]
    encxw0[:, 128:192] = encxw[:, 128:192]

    aps = np.zeros((128, 14), np.float32)
    aps[0:64, 0] = 0.2 * enc_b[0:64] + 0.5          # enc i
    aps[64:128, 0] = 0.2 * enc_b[64:128] + 0.5      # enc f
    aps[0:64, 1] = 0.2 * dec_b[64:128] + 0.5        # dec f (chunk A is [f|i])
    aps[64:128, 1] = 0.2 * dec_b[0:64] + 0.5        # dec i
    aps[0:64, 2] = enc_b[128:192]                   # enc c~ (tanh bias)
    aps[64:128, 2] = 0.2 * enc_b[192:256] + 0.5     # enc o
    aps[0:64, 3] = dec_b[128:192]
    aps[64:128, 3] = 0.2 * dec_b[192:256] + 0.5
    aps[0:64, 4] = s_e
    aps[0:64, 5] = t_e
    aps[0:64, 6] = s_d
    aps[0:64, 7] = t_d
    aps[0:64, 8] = 0.2 * enc_b[0:64] + 0.5          # ts0 enc i
    aps[64:128, 8] = 0.2 * enc_b[192:256] + 0.5     # ts0 enc o
    aps[0:64, 9] = enc_b[128:192]                   # ts0 enc c~ (tanh bias)
    aps[0:64, 10] = 0.2 * dec_b[0:64] + 0.5         # ts0 dec i
    aps[64:128, 10] = 0.2 * dec_b[192:256] + 0.5    # ts0 dec o
    aps[0:64, 11] = dec_b[128:192]                  # ts0 dec c~ (tanh bias)
    # ts0 pair-packing: cc tanh acts cover both partition halves (odd chunk's
    # cc lands at 64:128), and odd chunks use swapped [o|i] gate layouts
    aps[64:128, 9] = enc_b[128:192]                 # ts0 enc c~ odd half
    aps[64:128, 11] = dec_b[128:192]                # ts0 dec c~ odd half
    aps[0:64, 12] = 0.2 * enc_b[192:256] + 0.5      # ts0 enc odd o
    aps[64:128, 12] = 0.2 * enc_b[0:64] + 0.5       # ts0 enc odd i
    aps[0:64, 13] = 0.2 * dec_b[192:256] + 0.5      # ts0 dec odd o
    aps[64:128, 13] = 0.2 * dec_b[0:64] + 0.5       # ts0 dec odd i
    bout = np.tile(np.asarray(inputs["out_b"], np.float32), T).reshape(30, 1)
    # odd-chunk swapped gate layouts for the ts0 cc pair-packing
    encxw0o = np.concatenate([encxw0[:, 64:128], encxw0[:, 0:64]], axis=1)
    decw0o = np.concatenate([decw0[:, :, 64:128], decw0[:, :, 0:64]], axis=2)

    shared = {
        "encw": encw.reshape(128, 5 * 256).astype(BFP),
        "decw0": decw0.reshape(128, 5 * 256).astype(BFP),
        "encxw0": encxw0.astype(BFP),
        "encxw0o": encxw0o.astype(BFP),
        "decw0o": decw0o.reshape(128, 5 * 128).astype(BFP),
        "decw": decw.reshape(128, 9 * 256).astype(BFP),
        "w3": w3.reshape(128, 45 * 30).astype(BFP),
        "aps": aps,
        "bout": bout,
    }
    return [dict(shared, xim=np.ascontiguousarray(xim[c])) for c in range(8)]


_CACHE = {}


def kernel(**inputs):
    if "nc" not in _CACHE:
        _CACHE["nc"] = _build()
    nc = _CACHE["nc"]
    in_maps = _prep(inputs)
    from concourse.bass_utils import run_bass_kernel_spmd
    res = run_bass_kernel_spmd(nc, in_maps, core_ids=list(range(8)))
    kernel.last_exec_ns = res.exec_time_ns
    y = np.stack([
        np.asarray(res.results[c]["y"], np.float32)
        .reshape(T, 3, H, W).transpose(0, 2, 3, 1)
        for c in range(8)
    ])
    return y
